# revision 1
# baseline (speedup 1.0000x reference)
"""Trainium2 Bass kernel for nn_Attention_47313359733175.

Vector-neuron style attention: B=8, C=128, N=1024, H=8 heads.
  q/k/v = VNLinear(W, x)  : (B,384,3,N), reshaped to heads of 144 features
  attn  = softmax(q k^T / sqrt(48)), out = VNLinear(Wo, attn v)

Sharding: pure data-parallel over the batch dim; core i computes batch i.

Per-core plan (all on-chip after one input DMA):
  - Q/K projected into a 64-padded head-pair layout: chunk j holds heads
    2j (partitions 0:48) and 2j+1 (partitions 64:112), zero padding between.
    Contraction feature blocks then sit at 32-aligned partition bases, so
    scores S^T = K_blk^T Q_blk run as row-paired (even/odd head) K=64
    matmuls accumulating over the 3 vector components.  K projections are
    computed once; Q is projected per query-half on demand.
  - exp on ScalarE straight out of PSUM with the 1/sqrt(48) scale folded in.
    Weights are ~0.05-scale so scores are O(1): no max subtraction needed.
    ScalarE runs *only* exp + output drains; all other copies are on DVE.
  - V is projected transposed (sequence on partitions) with lhsT = x-slices,
    into per-head flat segments [feats 0-127][ones][pad][feats 128-143].
    The ones column makes the attn*V matmul also produce softmax row-sums,
    landing at psum partition 0.
  - U^T = V_seg^T E accumulates over key chunks in PSUM (M=128 + M=32).
    The inner loop is software-pipelined: U matmuls lag the score matmuls
    by one key chunk so the exp latency never stalls the PE.
  - Row-sum reciprocal broadcasts across partitions via the (otherwise idle)
    GPSIMD partition_broadcast custom op; the output projection uses
    host-precomputed zero-padded lhsT blocks so every psum write is at
    partition base 0 (this walrus rejects any other matmul dst base).
  - fp32r (11-bit mantissa, single-pass) matmuls everywhere: 4x the fp32 PE
    rate; inputs are pre-rounded on the host / by the producing engines.
"""

import os
import sys

sys.path.insert(0, "/opt/trn_rl_repo")

import numpy as np
from contextlib import ExitStack

import concourse.bass as bass
import concourse.bacc as bacc
import concourse.mybir as mybir
import concourse.tile as tile
from concourse.bass import ts, ds
from concourse.bass_utils import run_bass_kernel_spmd

P = 128          # partitions
N = 1024         # sequence length
C = 128          # input channels
F = 384          # projected channels (3C)
NH = 8           # heads
FH = 48          # channels per head
D3 = 3           # vector components
SEG = 160        # per-head V segment: [feats 0-127][ones][15 pad][feats 128-143]
VW = SEG * NH    # 1280
NCORES = 8
SCALE = float(FH) ** -0.5
PACK1W = D3 * N + 4 * P + 4 * P + F  # X, WqT, WkT, WvT = 4480
PACK2W = NH * 4 * P                  # output-projection lhsT blocks = 4096

F32 = mybir.dt.float32
# matmul compute dtype: float32r = single-pass reduced-precision fp32 (4x
# faster than true fp32 on the PE).  Overridable for accuracy experiments.
MM_DT = mybir.dt.float32r if os.environ.get("KERN_MM_DT", "f32r") == "f32r" else F32
DT_R = MM_DT  # dtype of tensors feeding matmuls


def _round_f32r(a):
    """Round to fp32r (8-bit exp, 11-bit mantissa) with round-to-nearest-even."""
    a = np.ascontiguousarray(a, np.float32)
    if MM_DT == F32:
        return a
    u = a.view(np.uint32).copy()
    u += np.uint32(0x7FF) + ((u >> np.uint32(12)) & np.uint32(1))
    u &= np.uint32(0xFFFFF000)
    return u.view(np.float32)


def _build_program():
    nc = bacc.Bacc(
        "TRN2", target_bir_lowering=False, debug=False, enable_asserts=False
    )

    packed = nc.dram_tensor("packed", (P, PACK1W), DT_R, kind="ExternalInput")
    wfin = nc.dram_tensor("wfin", (P, PACK2W), DT_R, kind="ExternalInput")
    out = nc.dram_tensor("out", (C, D3, N), F32, kind="ExternalOutput")

    with tile.TileContext(nc) as tc:
        with ExitStack() as ctx:
            const = ctx.enter_context(tc.tile_pool(name="const", bufs=1))
            vpool = ctx.enter_context(tc.tile_pool(name="vpool", bufs=1))
            kpool = ctx.enter_context(tc.tile_pool(name="kpool", bufs=1))
            qpp = ctx.enter_context(tc.tile_pool(name="qpp", bufs=2))
            epool = ctx.enter_context(tc.tile_pool(name="epool", bufs=8))
            uscp = ctx.enter_context(tc.tile_pool(name="uscp", bufs=6))
            rrp = ctx.enter_context(tc.tile_pool(name="rrp", bufs=4))
            # PSUM budget: 2 + 4 + 2 = 8 banks exactly.
            pps = ctx.enter_context(tc.tile_pool(name="pps", bufs=2, space="PSUM"))
            ppu = ctx.enter_context(tc.tile_pool(name="ppu", bufs=4, space="PSUM"))
            ppo = ctx.enter_context(tc.tile_pool(name="ppo", bufs=2, space="PSUM"))

            PK = const.tile([P, PACK1W], DT_R, name="PK")
            cuts = [0, F + N, F + N + 8 * P, F + 2 * N + 8 * P, PACK1W]
            for ci_ in range(4):
                nc.sync.dma_start(
                    PK[:, cuts[ci_] : cuts[ci_ + 1]],
                    packed.ap()[:, cuts[ci_] : cuts[ci_ + 1]],
                )
            WVT = PK[:, 0:F]
            Xd = [
                PK[:, F : F + N],
                PK[:, F + N + 8 * P : F + 2 * N + 8 * P],
                PK[:, F + 2 * N + 8 * P : F + 3 * N + 8 * P],
            ]
            WKT = PK[:, F + N : F + N + 4 * P]
            WQT = PK[:, F + N + 4 * P : F + N + 8 * P]
            WFT = const.tile([P, NH, 4, P], DT_R, name="WFT")
            nc.sync.dma_start(
                WFT[:], wfin.ap().rearrange("p (h j q) -> p h j q", h=NH, j=4)
            )
            OutSB = const.tile([P, D3, N], F32, name="OutSB")

            # ---- V projection: V_seq[m][:, seg(h)] = (x[:, d, m-slice]^T Wv^T)
            Vseq = [
                vpool.tile([P, VW], DT_R, name=f"vs{m}", tag=f"vs{m}")
                for m in range(8)
            ]
            # copies alternate DVE / ScalarE so neither engine gates the
            # prologue; V and K projections are interleaved for pipelining
            cp = [nc.vector.tensor_copy, lambda out, in_: nc.scalar.copy(out=out, in_=in_)]
            ci = [0]

            def copy_alt(out, in_):
                cp[ci[0] & 1](out=out, in_=in_)
                ci[0] += 1

            Kps = [
                kpool.tile([P, D3, N], DT_R, name=f"kp{pr}", tag=f"kp{pr}")
                for pr in range(4)
            ]
            vrs = []
            for m in range(8):
                vrs.append(Vseq[m].rearrange("p (h s) -> p h s", s=SEG))
                vu = Vseq[m].bitcast(mybir.dt.uint32).rearrange(
                    "p (h s) -> p h s", s=SEG
                )
                nc.vector.memset(vu[:, :, 129:144], 0)
                nc.vector.memset(vu[:, :, 128], 0x3F800000)
            def v_proj(m, d, pool, tag, eng_copy):
                vr = vrs[m]
                pv = pool.tile([P, F], F32, name=f"pv{m}{d}", tag=tag)
                nc.tensor.matmul(
                    pv[:], lhsT=Xd[d][:, ts(m, P)], rhs=WVT[:],
                    start=True, stop=True,
                )
                pvh = pv.rearrange("p (h f) -> p h f", f=FH)
                if d < 2:
                    eng_copy(out=vr[:, :, 48 * d : 48 * d + 48], in_=pvh)
                else:
                    # d2 feats split around the [ones|pad] block
                    eng_copy(out=vr[:, :, 96:128], in_=pvh[:, :, 0:32])
                    eng_copy(out=vr[:, :, 144:160], in_=pvh[:, :, 32:48])

            def k_proj(pr, dk, half, pool, tag, eng_copy):
                pk = pool.tile([P, 512], F32, name=f"pk{pr}{dk}{half}", tag=tag)
                nc.tensor.matmul(
                    pk[:], lhsT=WKT[:, ts(pr, P)], rhs=Xd[dk][:, ts(half, 512)],
                    start=True, stop=True,
                )
                eng_copy(out=Kps[pr][:, dk, ts(half, 512)], in_=pk[:])

            # prologue: V chunks 0-4 and K pairs 0-1; the rest interleaves
            # into the first nchunk's key loops (their consumers come late).
            kjobs = [(pr, dk, half) for pr in (0, 1, 2, 3) for dk in range(D3)
                     for half in range(2)]
            ki = 0
            for d in range(D3):
                for m in range(8):
                    v_proj(m, d, ppu, "pu", copy_alt)
                    if d >= 1 and ki < len(kjobs):
                        k_proj(*kjobs[ki], ppu, "pu", copy_alt)
                        ki += 1
                        if d == 2 and ki < len(kjobs):
                            k_proj(*kjobs[ki], ppu, "pu", copy_alt)
                            ki += 1
            while ki < len(kjobs):
                k_proj(*kjobs[ki], ppu, "pu", copy_alt)
                ki += 1


            # ---- main: 2 halves of the query dim, 4 head pairs
            def q_proj(nch_, pr):
                Qp = qpp.tile([P, D3, 512], DT_R, name=f"qp{nch_}{pr}", tag="qp")
                for d in range(D3):
                    pq = pps.tile([P, 512], F32, name=f"pq{nch_}{pr}{d}", tag="ps")
                    nc.tensor.matmul(
                        pq[:], lhsT=WQT[:, ts(pr, P)],
                        rhs=Xd[d][:, ds(512 * nch_, 512)],
                        start=True, stop=True,
                    )
                    nc.vector.tensor_copy(out=Qp[:, d, :], in_=pq[:])
                return Qp

            Qnext = q_proj(0, 0)
            for nch in range(2):
                OUTP = [
                    ppo.tile([P, 512], F32, name=f"op{nch}{d}", tag="po")
                    for d in range(2)
                ]
                for pair in range(4):
                    Kp = Kps[pair]
                    Qp = Qnext

                    # attention for heads (2*pair, 2*pair+1); U matmuls lag the
                    # scores by one key chunk so exp latency stays off the PE
                    pA = [
                        ppu.tile([P, 512], F32, name=f"pa{nch}{pair}{i}", tag="pu")
                        for i in range(2)
                    ]
                    pB = [
                        ppu.tile([P, 512], F32, name=f"pb{nch}{pair}{i}", tag="pu")
                        for i in range(2)
                    ]

                    def u_mms(m, Em):
                        for i in range(2):
                            h = 2 * pair + i
                            nc.tensor.matmul(
                                pA[i][:],
                                lhsT=Vseq[m][:, SEG * h : SEG * h + 128],
                                rhs=Em[i][:],
                                start=(m == 0), stop=(m == 7),
                            )
                            nc.tensor.matmul(
                                pB[i][0:32, :],
                                lhsT=Vseq[m][:, SEG * h + 128 : SEG * h + 160],
                                rhs=Em[i][:],
                                start=(m == 0), stop=(m == 7),
                            )

                    Eq = []
                    for m in range(8):
                        pS = [
                            pps.tile(
                                [P, 512], F32, name=f"s{nch}{pair}{m}{i}", tag="ps"
                            )
                            for i in range(2)
                        ]
                        for d in range(D3):
                            for i in range(2):
                                blk = slice(64 * i, 64 * i + 64)
                                nc.tensor.matmul(
                                    pS[i][:],
                                    lhsT=Kp[blk, d, ts(m, P)],
                                    rhs=Qp[blk, d, :],
                                    start=(d == 0), stop=(d == D3 - 1),
                                )
                        Em = []
                        for i in range(2):
                            E = epool.tile(
                                [P, 512], DT_R, name=f"e{nch}{pair}{m}{i}", tag="e"
                            )
                            nc.scalar.activation(
                                E[:], pS[i][:], mybir.ActivationFunctionType.Exp,
                                scale=SCALE,
                            )
                            Em.append(E)
                        Eq.append(Em)
                        if m >= 2:
                            u_mms(m - 2, Eq[m - 2])
                    u_mms(6, Eq[6])
                    u_mms(7, Eq[7])
                    if pair < 3:
                        Qnext = q_proj(nch, pair + 1)
                    elif nch == 0:
                        Qnext = q_proj(1, 0)

                    # normalize + output projection: both heads' reciprocal
                    # + broadcast first, then the dependent scale/projections
                    Rsbs = []
                    for i in range(2):
                        rr = rrp.tile([P, 512], F32, name=f"rr{nch}{pair}{i}", tag="rr")
                        nc.vector.reciprocal(out=rr[0:1, :], in_=pB[i][0:1, :])
                        Rsb = rrp.tile([P, 512], F32, name=f"rs{nch}{pair}{i}", tag="rs")
                        nc.gpsimd.partition_broadcast(Rsb[:], rr[0:1, :])
                        Rsbs.append(Rsb)
                    for i in range(2):
                        h = 2 * pair + i
                        Rsb = Rsbs[i]
                        UA = uscp.tile([P, 512], DT_R, name=f"ua{nch}{pair}{i}", tag="ua")
                        nc.vector.tensor_mul(out=UA[:], in0=pA[i][:], in1=Rsb[:])
                        UB = uscp.tile([P, 512], DT_R, name=f"ub{nch}{pair}{i}", tag="ub")
                        nc.vector.tensor_mul(
                            out=UB[0:32, :], in0=pB[i][0:32, :], in1=Rsb[0:32, :],
                        )
                        first = pair == 0 and i == 0
                        last = pair == 3 and i == 1
                        for d in range(2):
                            nc.tensor.matmul(
                                OUTP[d][:], lhsT=WFT[:, h, d, :], rhs=UA[:],
                                start=first, stop=last,
                            )
                        # d2 accumulates in SBUF (PSUM bank budget)
                        pD2 = ppu.tile([P, 512], F32, name=f"pd{nch}{pair}{i}", tag="pu")
                        nc.tensor.matmul(
                            pD2[:], lhsT=WFT[:, h, 2, :], rhs=UA[:],
                            start=True, stop=False,
                        )
                        nc.tensor.matmul(
                            pD2[:], lhsT=WFT[0:32, h, 3, :], rhs=UB[0:32, :],
                            start=False, stop=True,
                        )
                        osl = OutSB[:, 2, ds(512 * nch, 512)]
                        if first:
                            nc.vector.tensor_copy(out=osl, in_=pD2[:])
                        else:
                            nc.vector.tensor_add(out=osl, in0=osl, in1=pD2[:])
                        if last:
                            nc.sync.dma_start(
                                out.ap()[:, 2, ds(512 * nch, 512)], osl
                            )

                for d in range(2):
                    nc.scalar.copy(out=OutSB[:, d, ds(512 * nch, 512)], in_=OUTP[d][:])
                    nc.sync.dma_start(
                        out.ap()[:, d, ds(512 * nch, 512)],
                        OutSB[:, d, ds(512 * nch, 512)],
                    )

    nc.compile()
    return nc


def _prep_weights(Wq, Wk, Wv, Wo):
    def pad_qk(W):
        Wt = np.ascontiguousarray(W.T).astype(np.float32)  # (128 c, 384 o)
        arr = np.zeros((P, 4, P), np.float32)
        for h in range(NH):
            ch, half = divmod(h, 2)
            arr[:, ch, 64 * half : 64 * half + FH] = Wt[:, FH * h : FH * h + FH]
        return arr.reshape(P, 4 * P)

    WoT = np.ascontiguousarray(Wo.T).astype(np.float32)  # (384 o, 128 co)
    wf = np.zeros((P, NH, 4, P), np.float32)
    for h in range(NH):
        blk = WoT[FH * h : FH * h + FH]  # (48, 128)
        wf[0:48, h, 0] = blk
        wf[48:96, h, 1] = blk
        wf[96:128, h, 2] = blk[0:32]
        wf[16:32, h, 3] = blk[32:48]
    return (
        pad_qk(Wq),
        pad_qk(Wk),
        np.ascontiguousarray(Wv.T).astype(np.float32),
        np.ascontiguousarray(wf.reshape(P, NH * 4 * P)),
    )


_CACHED_NC = None


def _make_in_maps(vn_x, Wq, Wk, Wv, Wo):
    wqt, wkt, wvt, wf = (
        _round_f32r(w)
        for w in _prep_weights(
            np.asarray(Wq), np.asarray(Wk), np.asarray(Wv), np.asarray(Wo)
        )
    )
    vn_x = _round_f32r(np.asarray(vn_x))
    maps = []
    for b in range(NCORES):
        xb = vn_x[b]
        packed = np.concatenate(
            [wvt, xb[:, 0], wkt, wqt, xb[:, 1], xb[:, 2]], axis=1
        )
        assert packed.shape == (P, PACK1W)
        maps.append(
            {"packed": np.ascontiguousarray(packed), "wfin": wf}
        )
    return maps


def kernel(vn_x, Wq, Wk, Wv, Wo):
    global _CACHED_NC
    if _CACHED_NC is None:
        _CACHED_NC = _build_program()
    nc = _CACHED_NC

    in_maps = _make_in_maps(vn_x, Wq, Wk, Wv, Wo)
    res = run_bass_kernel_spmd(nc, in_maps, core_ids=list(range(NCORES)))
    out = np.stack([res.results[b]["out"] for b in range(NCORES)])
    return out



# revision 3
# speedup vs baseline: 1.0641x; 1.0641x over previous
"""Trainium2 Bass kernel v2 for nn_Attention_47313359733175.

Vector-neuron attention: B=8, C=128, N=1024, H=8 heads, head dim 144.
Data-parallel over batch; core b computes batch b.

Differences vs the prior baseline (199392 ns):
  - Scores use 2 matmuls per (head, key-chunk, q-half) tile instead of 3:
    Q/K are packed per head into a 128-row tile (d0|d1 feats + 32 zero pad
    rows) and a 64-row tile (d2 feats + 16 zero pad rows).  Pad rows are
    zeroed by DMA from a zeros input.  PE: 196608 -> 131072 cycles.
  - Projections run on dense 3x128-chunk weights (VNLinear weights are
    shared across the 3 vector components, so Wk.T / Wq.T DMA once).
  - All SBUF operands are bf16 (psum stays fp32): halves DMA + SBUF.
  - exp runs once per m-pair over a [128,1024] 2-bank psum tile.
"""

import sys

sys.path.insert(0, "/opt/trn_rl_repo")

import numpy as np
import ml_dtypes
from contextlib import ExitStack

import concourse.bass as bass
import concourse.bacc as bacc
import concourse.mybir as mybir
import concourse.tile as tile
from concourse.bass import ts, ds
from concourse.bass_utils import run_bass_kernel_spmd

P = 128
N = 1024
C = 128
F = 384
NH = 8
FH = 48
D3 = 3
SEG = 160        # V segment: [feats 0-127][ones][15 pad][feats 128-143]
VW = SEG * NH
NCORES = 8
SCALE = float(FH) ** -0.5

F32 = mybir.dt.float32
BF = mybir.dt.bfloat16

# packed layout (bf16): WKT | WQT | X0 | WVT | X1 | X2 | WFT
# WKT/WQT are head-pair padded: [128c, 4 pairs, (64+64)] with each head's 48
# output channels at a 64-aligned base (engine partition ranges must start
# 32-aligned, so scatter copies move whole 64-row blocks).
O_WKT = 0
O_WQT = 512
O_X0 = 1024
O_WVT = O_X0 + N
O_X1 = O_WVT + 384
O_X2 = O_X1 + N
O_WFT = O_X2 + N
PACKW = O_WFT + NH * 4 * P  # 4480 + 4096 = 8576


def _build_program():
    nc = bacc.Bacc(
        "TRN2", target_bir_lowering=False, debug=False, enable_asserts=False
    )

    packed = nc.dram_tensor("packed", (P, PACKW), BF, kind="ExternalInput")
    out = nc.dram_tensor("out", (C, D3, N), F32, kind="ExternalOutput")

    with tile.TileContext(nc) as tc:
        with ExitStack() as ctx:
            const = ctx.enter_context(tc.tile_pool(name="const", bufs=1))
            vpool = ctx.enter_context(tc.tile_pool(name="vpool", bufs=1))
            qkp = ctx.enter_context(tc.tile_pool(name="qkp", bufs=1))
            epool = ctx.enter_context(tc.tile_pool(name="epool", bufs=6))
            uscp = ctx.enter_context(tc.tile_pool(name="uscp", bufs=4))
            rrp = ctx.enter_context(tc.tile_pool(name="rrp", bufs=4))
            stgp = ctx.enter_context(tc.tile_pool(name="stgp", bufs=4))
            # PSUM: 2 + 2 + 2 + 2 = 8 banks
            ppBig = ctx.enter_context(tc.tile_pool(name="ppBig", bufs=2, space="PSUM"))
            ppU = ctx.enter_context(tc.tile_pool(name="ppU", bufs=2, space="PSUM"))
            ppB = ctx.enter_context(tc.tile_pool(name="ppB", bufs=2, space="PSUM"))
            ppO = ctx.enter_context(tc.tile_pool(name="ppO", bufs=2, space="PSUM"))

            PK = const.tile([P, PACKW], BF, name="PK")
            cuts = [0, O_X0, O_X0 + 512, O_WVT, O_X1, O_X2, O_WFT, PACKW]
            for ci in range(len(cuts) - 1):
                nc.sync.dma_start(
                    PK[:, cuts[ci] : cuts[ci + 1]],
                    packed.ap()[:, cuts[ci] : cuts[ci + 1]],
                )
            WKT = PK[:, O_WKT : O_WKT + 512]
            WQT = PK[:, O_WQT : O_WQT + 512]
            WVT = PK[:, O_WVT : O_WVT + F]
            Xd = [
                PK[:, O_X0 : O_X0 + N],
                PK[:, O_X1 : O_X1 + N],
                PK[:, O_X2 : O_X2 + N],
            ]
            WFT = PK[:, O_WFT:PACKW].rearrange("p (h j q) -> p h j q", h=NH, j=4)
            OutSB = const.tile([P, D3, N], F32, name="OutSB")

            # Q/K packed score operands (per head): A [128 rows: d0 48+16z |
            # d1 48+16z], B [64 rows: d2 48+16z]
            AK = qkp.tile([P, NH, N], BF, name="AK")
            AQ = qkp.tile([P, NH, N], BF, name="AQ")
            BK = qkp.tile([64, NH, N], BF, name="BK")
            BQ = qkp.tile([64, NH, N], BF, name="BQ")
            # pad rows (48:64, 112:128) are zero because the padded weight
            # rows produce zero psum that the 64-block copies carry along

            Vseq = [
                vpool.tile([P, VW], BF, name=f"vs{m}", tag=f"vs{m}")
                for m in range(8)
            ]
            vrs = []
            # prologue rotation: DVE 2/5, Pool 2/5, Act 1/5 (Act owns exp);
            # main-loop weave jobs avoid Act entirely.
            # psum-sourced copies may only run on DVE/Act (GPSIMD
            # cannot access PSUM -- walrus rejects it)
            engs = [nc.vector, nc.scalar]
            engs2 = [nc.vector]
            ei = [0]

            def rot_copy(out_, in_, main=False):
                es = engs2 if main else engs
                e = es[ei[0] % len(es)]
                ei[0] += 1
                if e is nc.scalar:
                    e.copy(out=out_, in_=in_)
                else:
                    e.tensor_copy(out=out_, in_=in_)

            for m in range(8):
                vrs.append(Vseq[m].rearrange("p (h s) -> p h s", s=SEG))
                vu = Vseq[m].bitcast(mybir.dt.uint16).rearrange(
                    "p (h s) -> p h s", s=SEG
                )
                nc.vector.memset(vu[:, :, 129:144], 0)
                nc.vector.memset(vu[:, :, 128], 0x3F80)

            pools4 = [(ppU, "pu"), (ppB, "pb"), (ppO, "po"), (ppBig, "ps")]
            pidx = [0]

            def next_pool():
                p_ = pools4[pidx[0] % 4]
                pidx[0] += 1
                return p_

            # ---- projection jobs ----
            def v_job(m, d, pool_tag=None, main=False):
                pool, tag = pool_tag or next_pool()
                pv = pool.tile([P, F], F32, name=f"pv{m}{d}", tag=tag)
                nc.tensor.matmul(
                    pv[:], lhsT=Xd[d][:, ts(m, P)], rhs=WVT[:],
                    start=True, stop=True,
                )
                pvh = pv.rearrange("p (h f) -> p h f", f=FH)
                vr = vrs[m]
                if d < 2:
                    rot_copy(vr[:, :, 48 * d : 48 * d + 48], pvh, main)
                else:
                    rot_copy(vr[:, :, 96:128], pvh[:, :, 0:32], main)
                    rot_copy(vr[:, :, 144:160], pvh[:, :, 32:48], main)

            def qk_job(which, d, j, half, pool_tag=None, main=False):
                # j is the head pair; chunk rows = [head 2j (48+16z) | head
                # 2j+1 (48+16z)] -> 64-aligned block copies into A/B tiles
                WT, A, B = (WKT, AK, BK) if which == "k" else (WQT, AQ, BQ)
                pool, tag = pool_tag or next_pool()
                pk = pool.tile([P, 512], F32, name=f"p{which}{d}{j}{half}", tag=tag)
                nc.tensor.matmul(
                    pk[:], lhsT=WT[:, ts(j, P)], rhs=Xd[d][:, ts(half, 512)],
                    start=True, stop=True,
                )
                sl = ts(half, 512)
                for i in range(2):
                    src = pk[64 * i : 64 * i + 64, :]
                    if d < 2:
                        rot_copy(A[64 * d : 64 * d + 64, 2 * j + i, sl], src, main)
                    else:
                        rot_copy(B[0:64, 2 * j + i, sl], src, main)

            # upfront: K and Q half-0 first (needed at main start), then V
            # woven with the second halves
            jobs = []
            for j in range(4):
                for d in range(D3):
                    jobs.append(("k", d, j, 0))
                    jobs.append(("k", d, j, 1))
                    jobs.append(("q", d, j, 0))
            for j in range(4):
                for d in range(D3):
                    jobs.append(("q", d, j, 1))
            vjobs = [(m, d) for m in range(8) for d in range(D3)]
            wove = []
            vi = 0
            for i, jb in enumerate(jobs):
                wove.append(("qk", jb))
                if i % 2 == 1 and vi < len(vjobs):
                    wove.append(("v", vjobs[vi])); vi += 1
            while vi < len(vjobs):
                wove.append(("v", vjobs[vi])); vi += 1
            for kind, jb in wove:
                if kind == "qk":
                    qk_job(*jb)
                else:
                    v_job(*jb)
            vweave = []
            q1jobs = []

            # ---- main loop ----
            ExpF = mybir.ActivationFunctionType.Exp
            for qh in range(2):
                OUTP = [
                    ppO.tile([P, 512], F32, name=f"op{qh}{d}", tag="po")
                    for d in range(2)
                ]
                qsl = ds(512 * qh, 512)
                for h in range(NH):
                    pA = ppU.tile([P, 512], F32, name=f"pa{qh}{h}", tag="pu")
                    pB = ppB.tile([32, 512], F32, name=f"pb{qh}{h}", tag="pb")
                    Epair = []

                    def u_mms(mm):
                        Em = Epair[mm][:]
                        nc.tensor.matmul(
                            pA[:],
                            lhsT=Vseq[mm][:, SEG * h : SEG * h + 128],
                            rhs=Em,
                            start=(mm == 0), stop=(mm == 7),
                        )
                        nc.tensor.matmul(
                            pB[:],
                            lhsT=Vseq[mm][:, SEG * h + 128 : SEG * h + 160],
                            rhs=Em,
                            start=(mm == 0), stop=(mm == 7),
                        )

                    for m in range(8):
                        if qh == 0 and h == 0:
                            # weave V projections in (Vseq[m] needed at m+2)
                            while vweave and vweave[0][0] <= m:
                                vm, vd = vweave.pop(0)
                                v_job(vm, vd, (ppBig, "ps"), main=True)
                        elif qh == 0 and 1 <= h <= 5 and m in (2, 6) and q1jobs:
                            d_, j_ = q1jobs.pop(0)
                            qk_job("q", d_, j_, 1, (ppBig, "ps"), main=True)
                        psS = ppBig.tile([P, 512], F32, name=f"s{qh}{h}{m}", tag="ps")
                        nc.tensor.matmul(
                            psS[:],
                            lhsT=AK[:, h, ts(m, P)], rhs=AQ[:, h, qsl],
                            start=True, stop=False,
                        )
                        nc.tensor.matmul(
                            psS[:],
                            lhsT=BK[0:64, h, ts(m, P)], rhs=BQ[0:64, h, qsl],
                            start=False, stop=True,
                        )
                        E = epool.tile([P, 512], BF, name=f"e{qh}{h}{m}", tag="e")
                        nc.scalar.activation(E[:], psS[:], ExpF, scale=SCALE)
                        Epair.append(E)
                        if m >= 2:
                            u_mms(m - 2)
                    u_mms(6)
                    u_mms(7)

                    # normalize + output projection
                    rr = rrp.tile([P, 512], F32, name=f"rr{qh}{h}", tag="rr")
                    nc.vector.reciprocal(out=rr[0:1, :], in_=pB[0:1, :])
                    Rsb = rrp.tile([P, 512], F32, name=f"rs{qh}{h}", tag="rs")
                    nc.gpsimd.partition_broadcast(Rsb[:], rr[0:1, :])
                    UA = uscp.tile([P, 512], BF, name=f"ua{qh}{h}", tag="ua")
                    nc.vector.tensor_mul(out=UA[:], in0=pA[:], in1=Rsb[:])
                    UB = uscp.tile([32, 512], BF, name=f"ub{qh}{h}", tag="ub")
                    nc.vector.tensor_mul(
                        out=UB[:], in0=pB[0:32, :], in1=Rsb[0:32, :],
                    )
                    first, last = h == 0, h == NH - 1
                    for d in range(2):
                        nc.tensor.matmul(
                            OUTP[d][:], lhsT=WFT[:, h, d, :], rhs=UA[:],
                            start=first, stop=last,
                        )
                    pD2 = ppU.tile([P, 512], F32, name=f"pd{qh}{h}", tag="pu")
                    nc.tensor.matmul(
                        pD2[:], lhsT=WFT[:, h, 2, :], rhs=UA[:],
                        start=True, stop=False,
                    )
                    nc.tensor.matmul(
                        pD2[:], lhsT=WFT[0:32, h, 3, :], rhs=UB[:],
                        start=False, stop=True,
                    )
                    osl = OutSB[:, 2, qsl]
                    if first:
                        nc.vector.tensor_copy(out=osl, in_=pD2[:])
                    else:
                        nc.vector.tensor_add(out=osl, in0=osl, in1=pD2[:])
                    if last:
                        nc.sync.dma_start(out.ap()[:, 2, qsl], osl)

                # drain d0/d1 on different engines + DMA queues in parallel
                nc.scalar.copy(out=OutSB[:, 0, qsl], in_=OUTP[0][:])
                nc.sync.dma_start(out.ap()[:, 0, qsl], OutSB[:, 0, qsl])
                nc.vector.tensor_copy(out=OutSB[:, 1, qsl], in_=OUTP[1][:])
                nc.scalar.dma_start(out.ap()[:, 1, qsl], OutSB[:, 1, qsl])

    nc.compile()
    return nc


def _prep_inputs(vn_x, Wq, Wk, Wv, Wo):
    bf = ml_dtypes.bfloat16
    WoT = np.ascontiguousarray(np.asarray(Wo, np.float32).T)  # (384, 128)
    wf = np.zeros((P, NH, 4, P), np.float32)
    for h in range(NH):
        blk = WoT[FH * h : FH * h + FH]  # (48, 128)
        wf[0:48, h, 0] = blk
        wf[48:96, h, 1] = blk
        wf[96:128, h, 2] = blk[0:32]
        wf[16:32, h, 3] = blk[32:48]
    def pad_pairs(W):
        Wt = np.asarray(W, np.float32).T  # (128, 384)
        arr = np.zeros((P, 4, 2, 64), np.float32)
        for h in range(NH):
            arr[:, h // 2, h % 2, 0:FH] = Wt[:, FH * h : FH * h + FH]
        return arr.reshape(P, 512)

    wkt = pad_pairs(Wk)
    wqt = pad_pairs(Wq)
    wvt = np.asarray(Wv, np.float32).T
    x = np.asarray(vn_x, np.float32)  # (B, C, 3, N)
    maps = []
    for b in range(NCORES):
        xb = x[b]
        packed = np.concatenate(
            [wkt, wqt, xb[:, 0], wvt, xb[:, 1], xb[:, 2],
             wf.reshape(P, NH * 4 * P)],
            axis=1,
        ).astype(bf)
        assert packed.shape == (P, PACKW)
        maps.append({"packed": np.ascontiguousarray(packed)})
    return maps


_CACHED_NC = None


def kernel(vn_x, Wq, Wk, Wv, Wo):
    global _CACHED_NC
    if _CACHED_NC is None:
        _CACHED_NC = _build_program()
    nc = _CACHED_NC
    in_maps = _prep_inputs(vn_x, Wq, Wk, Wv, Wo)
    res = run_bass_kernel_spmd(nc, in_maps, core_ids=list(range(NCORES)))
    return np.stack([res.results[b]["out"] for b in range(NCORES)])


# revision 4
# speedup vs baseline: 1.1977x; 1.1256x over previous
"""Trainium2 Bass kernel v2 for nn_Attention_47313359733175.

Vector-neuron attention: B=8, C=128, N=1024, H=8 heads, head dim 144.
Data-parallel over batch; core b computes batch b.

Differences vs the prior baseline (199392 ns):
  - Scores use 2 matmuls per (head, key-chunk, q-half) tile instead of 3:
    Q/K are packed per head into a 128-row tile (d0|d1 feats + 32 zero pad
    rows) and a 64-row tile (d2 feats + 16 zero pad rows).  Pad rows are
    zeroed by DMA from a zeros input.  PE: 196608 -> 131072 cycles.
  - Projections run on dense 3x128-chunk weights (VNLinear weights are
    shared across the 3 vector components, so Wk.T / Wq.T DMA once).
  - All SBUF operands are bf16 (psum stays fp32): halves DMA + SBUF.
  - exp runs once per m-pair over a [128,1024] 2-bank psum tile.
"""

import sys

sys.path.insert(0, "/opt/trn_rl_repo")

import numpy as np
import ml_dtypes
from contextlib import ExitStack

import concourse.bass as bass
import concourse.bacc as bacc
import concourse.mybir as mybir
import concourse.tile as tile
from concourse.bass import ts, ds
from concourse.bass_utils import run_bass_kernel_spmd

P = 128
N = 1024
C = 128
F = 384
NH = 8
FH = 48
D3 = 3
SEG = 160        # V segment: [feats 0-127][ones][15 pad][feats 128-143]
VW = SEG * NH
NCORES = 8
SCALE = float(FH) ** -0.5

F32 = mybir.dt.float32
BF = mybir.dt.bfloat16

# packed layout (bf16): WKT | WQT | X0 | WVT | X1 | X2 | WFT
# WKT/WQT are head-pair padded: [128c, 4 pairs, (64+64)] with each head's 48
# output channels at a 64-aligned base (engine partition ranges must start
# 32-aligned, so scatter copies move whole 64-row blocks).
O_WKT = 0
O_WQT = 512
O_X0 = 1024
O_WVT = O_X0 + N
O_X1 = O_WVT + 384
O_X2 = O_X1 + N
O_WFT = O_X2 + N
PACKW = O_WFT + NH * 4 * P  # 4480 + 4096 = 8576


def _build_program():
    nc = bacc.Bacc(
        "TRN2", target_bir_lowering=False, debug=False, enable_asserts=False
    )

    packed = nc.dram_tensor("packed", (P, PACKW), BF, kind="ExternalInput")
    out = nc.dram_tensor("out", (C, D3, N), F32, kind="ExternalOutput")

    with tile.TileContext(nc) as tc:
        with ExitStack() as ctx:
            const = ctx.enter_context(tc.tile_pool(name="const", bufs=1))
            vpool = ctx.enter_context(tc.tile_pool(name="vpool", bufs=1))
            qkp = ctx.enter_context(tc.tile_pool(name="qkp", bufs=1))
            epool = ctx.enter_context(tc.tile_pool(name="epool", bufs=16))
            utp = ctx.enter_context(tc.tile_pool(name="utp", bufs=8))
            ttp = ctx.enter_context(tc.tile_pool(name="ttp", bufs=8))
            rrp = ctx.enter_context(tc.tile_pool(name="rrp", bufs=8))
            # PSUM: 2 + 3 + 3 = 8 banks
            ppBig = ctx.enter_context(tc.tile_pool(name="ppBig", bufs=2, space="PSUM"))
            ppU = ctx.enter_context(tc.tile_pool(name="ppU", bufs=3, space="PSUM"))
            ppO = ctx.enter_context(tc.tile_pool(name="ppO", bufs=3, space="PSUM"))

            PK = const.tile([P, PACKW], BF, name="PK")
            cuts = [0, O_X0, O_X0 + 512, O_WVT, O_X1, O_X2, O_WFT, PACKW]
            for ci in range(len(cuts) - 1):
                nc.sync.dma_start(
                    PK[:, cuts[ci] : cuts[ci + 1]],
                    packed.ap()[:, cuts[ci] : cuts[ci + 1]],
                )
            WKT = PK[:, O_WKT : O_WKT + 512]
            WQT = PK[:, O_WQT : O_WQT + 512]
            WVT = PK[:, O_WVT : O_WVT + F]
            Xd = [
                PK[:, O_X0 : O_X0 + N],
                PK[:, O_X1 : O_X1 + N],
                PK[:, O_X2 : O_X2 + N],
            ]
            WFT = PK[:, O_WFT:PACKW].rearrange("p (h j q) -> p h j q", h=NH, j=4)
            OutSB = const.tile([P, D3, N], F32, name="OutSB")

            # Q/K packed score operands (per head): A [128 rows: d0 48+16z |
            # d1 48+16z], B [64 rows: d2 48+16z]
            AK = qkp.tile([P, NH, N], BF, name="AK")
            AQ = qkp.tile([P, NH, N], BF, name="AQ")
            BK = qkp.tile([64, NH, N], BF, name="BK")
            BQ = qkp.tile([64, NH, N], BF, name="BQ")
            # pad rows (48:64, 112:128) are zero because the padded weight
            # rows produce zero psum that the 64-block copies carry along

            Vseq = [
                vpool.tile([P, VW], BF, name=f"vs{m}", tag=f"vs{m}")
                for m in range(8)
            ]
            vrs = []
            # prologue rotation: DVE 2/5, Pool 2/5, Act 1/5 (Act owns exp);
            # main-loop weave jobs avoid Act entirely.
            # psum-sourced copies may only run on DVE/Act (GPSIMD
            # cannot access PSUM -- walrus rejects it)
            engs = [nc.vector, nc.scalar]
            engs2 = [nc.vector]
            ei = [0]

            def rot_copy(out_, in_, main=False):
                es = engs2 if main else engs
                e = es[ei[0] % len(es)]
                ei[0] += 1
                if e is nc.scalar:
                    e.copy(out=out_, in_=in_)
                else:
                    e.tensor_copy(out=out_, in_=in_)

            # Vseq segment layout (v3): [feats-a 0:128 | ones@128 | d2b
            # 129:145 | pad 145:160].  The U-form matmul streams cols 0:145
            # so one mm yields features, row-sum, and the d2 leftovers.
            for m in range(8):
                vrs.append(Vseq[m].rearrange("p (h s) -> p h s", s=SEG))
                vu = Vseq[m].bitcast(mybir.dt.uint16).rearrange(
                    "p (h s) -> p h s", s=SEG
                )
                nc.vector.memset(vu[:, :, 128], 0x3F80)

            pools3 = [(ppU, "pu"), (ppO, "po"), (ppBig, "ps")]
            pidx = [0]

            def next_pool():
                p_ = pools3[pidx[0] % len(pools3)]
                pidx[0] += 1
                return p_

            # ---- projection jobs ----
            def v_job(m, d, pool_tag=None, main=False):
                pool, tag = pool_tag or next_pool()
                pv = pool.tile([P, F], F32, name=f"pv{m}{d}", tag=tag)
                nc.tensor.matmul(
                    pv[:], lhsT=Xd[d][:, ts(m, P)], rhs=WVT[:],
                    start=True, stop=True,
                )
                pvh = pv.rearrange("p (h f) -> p h f", f=FH)
                vr = vrs[m]
                if d < 2:
                    rot_copy(vr[:, :, 48 * d : 48 * d + 48], pvh, main)
                else:
                    rot_copy(vr[:, :, 96:128], pvh[:, :, 0:32], main)
                    rot_copy(vr[:, :, 129:145], pvh[:, :, 32:48], main)

            def qk_job(which, d, j, half, pool_tag=None, main=False):
                # j is the head pair; chunk rows = [head 2j (48+16z) | head
                # 2j+1 (48+16z)] -> 64-aligned block copies into A/B tiles
                WT, A, B = (WKT, AK, BK) if which == "k" else (WQT, AQ, BQ)
                pool, tag = pool_tag or next_pool()
                pk = pool.tile([P, 512], F32, name=f"p{which}{d}{j}{half}", tag=tag)
                nc.tensor.matmul(
                    pk[:], lhsT=WT[:, ts(j, P)], rhs=Xd[d][:, ts(half, 512)],
                    start=True, stop=True,
                )
                sl = ts(half, 512)
                for i in range(2):
                    src = pk[64 * i : 64 * i + 64, :]
                    if d < 2:
                        rot_copy(A[64 * d : 64 * d + 64, 2 * j + i, sl], src, main)
                    else:
                        rot_copy(B[0:64, 2 * j + i, sl], src, main)

            # upfront: K and Q half-0 first (needed at main start), then V
            # woven with the second halves
            jobs = []
            for j in range(4):
                for d in range(D3):
                    jobs.append(("k", d, j, 0))
                    jobs.append(("k", d, j, 1))
                    jobs.append(("q", d, j, 0))
            for j in range(4):
                for d in range(D3):
                    jobs.append(("q", d, j, 1))
            vjobs = [(m, d) for m in range(8) for d in range(D3)]
            wove = []
            vi = 0
            for i, jb in enumerate(jobs):
                wove.append(("qk", jb))
                if i % 2 == 1 and vi < len(vjobs):
                    wove.append(("v", vjobs[vi])); vi += 1
            while vi < len(vjobs):
                wove.append(("v", vjobs[vi])); vi += 1
            for kind, jb in wove:
                if kind == "qk":
                    qk_job(*jb)
                else:
                    v_job(*jb)
            # ---- main loop (U-form) ----
            # per head: scores+exp; then the previous head's U block
            # (U-matmuls -> recip -> normalize-mul -> XBAR transposes); the
            # head-before-that's output projection (so transpose DMA latency
            # is fully hidden).
            ExpF = mybir.ActivationFunctionType.Exp
            mulengs = [nc.vector, nc.scalar]

            for qh in range(2):
                OUTP = [
                    ppO.tile([P, 512], F32, name=f"op{qh}{d}", tag="po")
                    for d in range(3)
                ]
                qsl = ds(512 * qh, 512)
                Es = {}
                Ts = {}
                utk = [0]

                def score_block(h):
                    Eh = []
                    for m in range(8):
                        psS = ppBig.tile([P, 512], F32, name=f"s{qh}{h}{m}", tag="ps")
                        nc.tensor.matmul(
                            psS[:],
                            lhsT=AK[:, h, ts(m, P)], rhs=AQ[:, h, qsl],
                            start=True, stop=False,
                        )
                        nc.tensor.matmul(
                            psS[:],
                            lhsT=BK[0:64, h, ts(m, P)], rhs=BQ[0:64, h, qsl],
                            start=False, stop=True,
                        )
                        E = epool.tile([P, 512], BF, name=f"e{qh}{h}{m}", tag="e")
                        nc.scalar.activation(E[:], psS[:], ExpF, scale=SCALE)
                        Eh.append(E)
                    Es[h] = Eh

                def u_block(h):
                    Eh = Es[h]
                    Th = []
                    for qc in range(4):
                        U = ppU.tile([P, 160], F32, name=f"u{qh}{h}{qc}", tag="pu")
                        for m in range(8):
                            nc.tensor.matmul(
                                U[:, 0:145],
                                lhsT=Eh[m][:, ts(qc, P)],
                                rhs=Vseq[m][:, SEG * h : SEG * h + 145],
                                start=(m == 0), stop=(m == 7),
                            )
                        rr = rrp.tile([P, 1], F32, name=f"rr{qh}{h}{qc}", tag="rr")
                        nc.vector.reciprocal(out=rr[:], in_=U[:, 128:129])
                        Ut = utp.tile([P, 192], BF, name=f"ut{qh}{h}{qc}", tag="ut")
                        # cols 145:192 feed the second XBAR transpose; zero
                        # them (gpsimd is idle) so its input stays finite
                        nc.gpsimd.memset(Ut[:, 145:192], 0)
                        eng = mulengs[(h + qc) % 2]
                        if eng is nc.scalar:
                            eng.activation(
                                Ut[:, 0:145], U[:, 0:145],
                                mybir.ActivationFunctionType.Copy, scale=rr,
                            )
                        else:
                            eng.tensor_scalar_mul(Ut[:, 0:145], U[:, 0:145], rr)
                        T1 = ttp.tile([P, 128], BF, name=f"t1{qh}{h}{qc}", tag="t1")
                        T2 = ttp.tile([P, 128], BF, name=f"t2{qh}{h}{qc}", tag="t2")
                        nc.sync.dma_start(T1[:], Ut[:, 0:128], transpose=True)
                        # T2 rows 64:96 = Ut cols 128:160 (rowsum, d2b)
                        nc.sync.dma_start(T2[:], Ut[:, 64:192], transpose=True)
                        Th.append((T1, T2))
                    Ts[h] = Th

                def out_block(h):
                    first, last = h == 0, h == NH - 1
                    for qc in range(4):
                        T1, T2 = Ts[h][qc]
                        st = first and qc == 0
                        sp = last and qc == 3
                        col = ts(qc, P)
                        for d in range(2):
                            nc.tensor.matmul(
                                OUTP[d][:, col], lhsT=WFT[:, h, d, :],
                                rhs=T1[:], start=st, stop=sp,
                            )
                        nc.tensor.matmul(
                            OUTP[2][:, col], lhsT=WFT[:, h, 2, :],
                            rhs=T1[:], start=st, stop=False,
                        )
                        nc.tensor.matmul(
                            OUTP[2][:, col], lhsT=WFT[64:96, h, 3, :],
                            rhs=T2[64:96, :], start=False, stop=sp,
                        )
                    del Ts[h]
                    if h >= 1:
                        del Es[h - 1]

                for h in range(NH):
                    score_block(h)
                    if h >= 1:
                        u_block(h - 1)
                    if h >= 2:
                        out_block(h - 2)
                u_block(NH - 1)
                out_block(NH - 2)
                out_block(NH - 1)

                # drain the three output components in parallel
                nc.scalar.copy(out=OutSB[:, 0, qsl], in_=OUTP[0][:])
                nc.sync.dma_start(out.ap()[:, 0, qsl], OutSB[:, 0, qsl])
                nc.vector.tensor_copy(out=OutSB[:, 1, qsl], in_=OUTP[1][:])
                nc.scalar.dma_start(out.ap()[:, 1, qsl], OutSB[:, 1, qsl])
                nc.vector.tensor_copy(out=OutSB[:, 2, qsl], in_=OUTP[2][:])
                nc.sync.dma_start(out.ap()[:, 2, qsl], OutSB[:, 2, qsl])

    nc.compile()
    return nc


def _prep_inputs(vn_x, Wq, Wk, Wv, Wo):
    bf = ml_dtypes.bfloat16
    WoT = np.ascontiguousarray(np.asarray(Wo, np.float32).T)  # (384, 128)
    wf = np.zeros((P, NH, 4, P), np.float32)
    for h in range(NH):
        blk = WoT[FH * h : FH * h + FH]  # (48, 128)
        wf[0:48, h, 0] = blk
        wf[48:96, h, 1] = blk
        wf[96:128, h, 2] = blk[0:32]
        # T2 rows 64:96 = Ut cols 128:160: row 64 = normalized row-sum,
        # rows 65:81 = d2 leftover feats
        wf[65:81, h, 3] = blk[32:48]
    def pad_pairs(W):
        Wt = np.asarray(W, np.float32).T  # (128, 384)
        arr = np.zeros((P, 4, 2, 64), np.float32)
        for h in range(NH):
            arr[:, h // 2, h % 2, 0:FH] = Wt[:, FH * h : FH * h + FH]
        return arr.reshape(P, 512)

    wkt = pad_pairs(Wk)
    wqt = pad_pairs(Wq)
    wvt = np.asarray(Wv, np.float32).T
    x = np.asarray(vn_x, np.float32)  # (B, C, 3, N)
    maps = []
    for b in range(NCORES):
        xb = x[b]
        packed = np.concatenate(
            [wkt, wqt, xb[:, 0], wvt, xb[:, 1], xb[:, 2],
             wf.reshape(P, NH * 4 * P)],
            axis=1,
        ).astype(bf)
        assert packed.shape == (P, PACKW)
        maps.append({"packed": np.ascontiguousarray(packed)})
    return maps


_CACHED_NC = None


def kernel(vn_x, Wq, Wk, Wv, Wo):
    global _CACHED_NC
    if _CACHED_NC is None:
        _CACHED_NC = _build_program()
    nc = _CACHED_NC
    in_maps = _prep_inputs(vn_x, Wq, Wk, Wv, Wo)
    res = run_bass_kernel_spmd(nc, in_maps, core_ids=list(range(NCORES)))
    return np.stack([res.results[b]["out"] for b in range(NCORES)])


# revision 6
# speedup vs baseline: 1.3232x; 1.1047x over previous
"""Trainium2 Bass kernel v2 for nn_Attention_47313359733175.

Vector-neuron attention: B=8, C=128, N=1024, H=8 heads, head dim 144.
Data-parallel over batch; core b computes batch b.

Differences vs the prior baseline (199392 ns):
  - Scores use 2 matmuls per (head, key-chunk, q-half) tile instead of 3:
    Q/K are packed per head into a 128-row tile (d0|d1 feats + 32 zero pad
    rows) and a 64-row tile (d2 feats + 16 zero pad rows).  Pad rows are
    zeroed by DMA from a zeros input.  PE: 196608 -> 131072 cycles.
  - Projections run on dense 3x128-chunk weights (VNLinear weights are
    shared across the 3 vector components, so Wk.T / Wq.T DMA once).
  - All SBUF operands are bf16 (psum stays fp32): halves DMA + SBUF.
  - exp runs once per m-pair over a [128,1024] 2-bank psum tile.
"""

import sys

sys.path.insert(0, "/opt/trn_rl_repo")

import numpy as np
import ml_dtypes
from contextlib import ExitStack

import concourse.bass as bass
import concourse.bacc as bacc
import concourse.mybir as mybir
import concourse.tile as tile
from concourse.bass import ts, ds
from concourse.bass_utils import run_bass_kernel_spmd

P = 128
N = 1024
C = 128
F = 384
NH = 8
FH = 48
D3 = 3
SEG = 160        # V segment: [feats 0-127][ones][15 pad][feats 128-143]
VW = SEG * NH
NCORES = 8
SCALE = float(FH) ** -0.5

F32 = mybir.dt.float32
BF = mybir.dt.bfloat16

# packed layout (bf16): WKT | X0 | WQT | WVT | X1 | X2 | WFT
# WKT/WQT are head-pair padded: [128c, 4 pairs, (64+64)] with each head's 48
# output channels at a 64-aligned base (engine partition ranges must start
# 32-aligned, so scatter copies move whole 64-row blocks).
O_WKT = 0
O_X0 = 512
O_WQT = O_X0 + N
O_WVT = O_WQT + 512
O_X1 = O_WVT + 384
O_X2 = O_X1 + N
O_WFT = O_X2 + N
PACKW = O_WFT + NH * 4 * P  # 4480 + 4096 = 8576


def _build_program():
    nc = bacc.Bacc(
        "TRN2", target_bir_lowering=False, debug=False, enable_asserts=False
    )

    packed = nc.dram_tensor("packed", (P, PACKW), BF, kind="ExternalInput")
    out = nc.dram_tensor("out", (C, D3, N), F32, kind="ExternalOutput")

    with tile.TileContext(nc) as tc:
        with ExitStack() as ctx:
            const = ctx.enter_context(tc.tile_pool(name="const", bufs=1))
            vpool = ctx.enter_context(tc.tile_pool(name="vpool", bufs=1))
            qkp = ctx.enter_context(tc.tile_pool(name="qkp", bufs=1))
            epool = ctx.enter_context(tc.tile_pool(name="epool", bufs=16))
            utp = ctx.enter_context(tc.tile_pool(name="utp", bufs=8))
            ttp = ctx.enter_context(tc.tile_pool(name="ttp", bufs=8))
            rrp = ctx.enter_context(tc.tile_pool(name="rrp", bufs=8))
            # PSUM: 2 + 3 + 3 = 8 banks
            ppBig = ctx.enter_context(tc.tile_pool(name="ppBig", bufs=2, space="PSUM"))
            ppU = ctx.enter_context(tc.tile_pool(name="ppU", bufs=3, space="PSUM"))
            ppO = ctx.enter_context(tc.tile_pool(name="ppO", bufs=3, space="PSUM"))

            PK = const.tile([P, PACKW], BF, name="PK")
            cuts = [0, O_X0, O_X0 + 512, O_WQT, O_WVT, O_X1, O_X2, O_WFT, PACKW]
            for ci in range(len(cuts) - 1):
                nc.sync.dma_start(
                    PK[:, cuts[ci] : cuts[ci + 1]],
                    packed.ap()[:, cuts[ci] : cuts[ci + 1]],
                )
            WKT = PK[:, O_WKT : O_WKT + 512]
            WQT = PK[:, O_WQT : O_WQT + 512]
            WVT = PK[:, O_WVT : O_WVT + F]
            Xd = [
                PK[:, O_X0 : O_X0 + N],
                PK[:, O_X1 : O_X1 + N],
                PK[:, O_X2 : O_X2 + N],
            ]
            WFT = PK[:, O_WFT:PACKW].rearrange("p (h j q) -> p h j q", h=NH, j=4)
            OutSB = const.tile([P, D3, N], F32, name="OutSB")

            # Q/K packed score operands (per head): A [128 rows: d0 48+16z |
            # d1 48+16z], B [64 rows: d2 48+16z]
            AK = qkp.tile([P, NH, N], BF, name="AK")
            AQ = qkp.tile([P, NH, N], BF, name="AQ")
            BK = qkp.tile([64, NH, N], BF, name="BK")
            BQ = qkp.tile([64, NH, N], BF, name="BQ")
            # pad rows (48:64, 112:128) are zero because the padded weight
            # rows produce zero psum that the 64-block copies carry along

            Vseq = [
                vpool.tile([P, VW], BF, name=f"vs{m}", tag=f"vs{m}")
                for m in range(8)
            ]
            vrs = []
            # prologue rotation: DVE 2/5, Pool 2/5, Act 1/5 (Act owns exp);
            # main-loop weave jobs avoid Act entirely.
            # psum-sourced copies may only run on DVE/Act (GPSIMD cannot
            # access PSUM).  Pre-main Act is exp-free so copies split 50/50;
            # in the main loop exp loads Act heavily, so copies lean DVE.
            engs_pro = [nc.vector, nc.scalar]
            engs_main = [nc.vector, nc.vector, nc.scalar]
            in_main = [False]
            ei = [0]

            def rot_copy(out_, in_, main=False):
                es = engs_main if in_main[0] else engs_pro
                e = es[ei[0] % len(es)]
                ei[0] += 1
                if e is nc.scalar:
                    e.copy(out=out_, in_=in_)
                else:
                    e.tensor_copy(out=out_, in_=in_)

            # Vseq segment layout (v3): [feats-a 0:128 | ones@128 | d2b
            # 129:145 | pad 145:160].  The U-form matmul streams cols 0:145
            # so one mm yields features, row-sum, and the d2 leftovers.
            for m in range(8):
                vrs.append(Vseq[m].rearrange("p (h s) -> p h s", s=SEG))
                vu = Vseq[m].bitcast(mybir.dt.uint16).rearrange(
                    "p (h s) -> p h s", s=SEG
                )
                nc.vector.memset(vu[:, :, 128], 0x3F80)

            pools3 = [(ppU, "pu"), (ppO, "po"), (ppBig, "ps")]
            pidx = [0]

            def next_pool():
                p_ = pools3[pidx[0] % len(pools3)]
                pidx[0] += 1
                return p_

            # ---- projection jobs ----
            def v_job(m, d, pool_tag=None, main=False):
                pool, tag = pool_tag or next_pool()
                pv = pool.tile([P, F], F32, name=f"pv{m}{d}", tag=tag)
                nc.tensor.matmul(
                    pv[:], lhsT=Xd[d][:, ts(m, P)], rhs=WVT[:],
                    start=True, stop=True,
                )
                pvh = pv.rearrange("p (h f) -> p h f", f=FH)
                vr = vrs[m]
                if d < 2:
                    rot_copy(vr[:, :, 48 * d : 48 * d + 48], pvh, main)
                else:
                    rot_copy(vr[:, :, 96:128], pvh[:, :, 0:32], main)
                    rot_copy(vr[:, :, 129:145], pvh[:, :, 32:48], main)

            def qk_job(which, d, j, half, pool_tag=None, main=False):
                # j is the head pair; chunk rows = [head 2j (48+16z) | head
                # 2j+1 (48+16z)] -> 64-aligned block copies into A/B tiles
                WT, A, B = (WKT, AK, BK) if which == "k" else (WQT, AQ, BQ)
                pool, tag = pool_tag or next_pool()
                pk = pool.tile([P, 512], F32, name=f"p{which}{d}{j}{half}", tag=tag)
                nc.tensor.matmul(
                    pk[:], lhsT=WT[:, ts(j, P)], rhs=Xd[d][:, ts(half, 512)],
                    start=True, stop=True,
                )
                sl = ts(half, 512)
                for i in range(2):
                    src = pk[64 * i : 64 * i + 64, :]
                    if d < 2:
                        rot_copy(A[64 * d : 64 * d + 64, 2 * j + i, sl], src, main)
                    else:
                        rot_copy(B[0:64, 2 * j + i, sl], src, main)

            # upfront: only what unit 0/1 needs -- K pairs 0-1 (both
            # halves), Q half-0 pairs 0-1, V for keys 0:768.  Everything
            # else is woven into the main loop ahead of its deadline.
            jobs = []
            for j in range(2):
                for d in range(D3):
                    jobs.append(("k", d, j, 0))
                    jobs.append(("k", d, j, 1))
                    jobs.append(("q", d, j, 0))
            vjobs = [(m, d) for m in range(6) for d in range(D3)]
            wove = []
            vi = 0
            for i, jb in enumerate(jobs):
                wove.append(("qk", jb))
                if i % 2 == 1 and vi < len(vjobs):
                    wove.append(("v", vjobs[vi])); vi += 1
            while vi < len(vjobs):
                wove.append(("v", vjobs[vi])); vi += 1
            for kind, jb in wove:
                if kind == "qk":
                    qk_job(*jb)
                else:
                    v_job(*jb)
            # woven into the first two score blocks via the (still free) ppO
            # ring: remaining V, then K/Q0 pairs 2-3 (deadlines i=4/i=6)
            vweave = [("v", m, d) for m in range(6, 8) for d in range(D3)]
            for j in (2, 3):
                for d in range(D3):
                    vweave.append(("k", d, j, 0))
                    vweave.append(("k", d, j, 1))
                    vweave.append(("q", d, j, 0))
            # woven one-per-unit from i=2: Q half-1 (pair j needed at i=8+2j)
            q1jobs = [("q", d, j, 1) for j in range(4) for d in range(D3)]
            # ---- main loop (U-form) ----
            # per head: scores+exp; then the previous head's U block
            # (U-matmuls -> recip -> normalize-mul -> XBAR transposes); the
            # head-before-that's output projection (so transpose DMA latency
            # is fully hidden).
            ExpF = mybir.ActivationFunctionType.Exp
            mulengs = [nc.vector, nc.vector, nc.vector, nc.scalar]
            units = [(qh, h) for qh in range(2) for h in range(NH)]
            Es = {}
            Ts = {}
            OUTPs = {}

            def score_block(qh, h):
                qsl = ds(512 * qh, 512)
                Eh = []
                for m in range(8):
                    for _ in range(2):
                        if qh == 0 and h < 2 and vweave:
                            jb = vweave.pop(0)
                            if jb[0] == "v":
                                v_job(jb[1], jb[2], pool_tag=(ppO, "po"))
                            else:
                                qk_job(*jb, pool_tag=(ppO, "po"))
                    psS = ppBig.tile([P, 512], F32, name=f"s{qh}{h}{m}", tag="ps")
                    nc.tensor.matmul(
                        psS[:],
                        lhsT=AK[:, h, ts(m, P)], rhs=AQ[:, h, qsl],
                        start=True, stop=False,
                    )
                    nc.tensor.matmul(
                        psS[:],
                        lhsT=BK[0:64, h, ts(m, P)], rhs=BQ[0:64, h, qsl],
                        start=False, stop=True,
                    )
                    E = epool.tile([P, 512], BF, name=f"e{qh}{h}{m}", tag="e")
                    nc.scalar.activation(E[:], psS[:], ExpF, scale=SCALE)
                    Eh.append(E)
                Es[(qh, h)] = Eh

            def u_block(qh, h, qcs=range(4)):
                Eh = Es[(qh, h)]
                Th = Ts.setdefault((qh, h), [])
                for qc in qcs:
                    U = ppU.tile([P, 160], F32, name=f"u{qh}{h}{qc}", tag="pu")
                    for m in range(8):
                        nc.tensor.matmul(
                            U[:, 0:145],
                            lhsT=Eh[m][:, ts(qc, P)],
                            rhs=Vseq[m][:, SEG * h : SEG * h + 145],
                            start=(m == 0), stop=(m == 7),
                        )
                    rr = rrp.tile([P, 1], F32, name=f"rr{qh}{h}{qc}", tag="rr")
                    nc.vector.reciprocal(out=rr[:], in_=U[:, 128:129])
                    Ut = utp.tile([P, 192], BF, name=f"ut{qh}{h}{qc}", tag="ut")
                    # cols 145:192 feed the second XBAR transpose; zero them
                    # (gpsimd is idle) so its input stays finite
                    nc.gpsimd.memset(Ut[:, 145:192], 0)
                    eng = mulengs[(4 * h + qc) % len(mulengs)]
                    if eng is nc.scalar:
                        eng.activation(
                            Ut[:, 0:145], U[:, 0:145],
                            mybir.ActivationFunctionType.Copy, scale=rr,
                        )
                    else:
                        eng.tensor_scalar_mul(Ut[:, 0:145], U[:, 0:145], rr)
                    T1 = ttp.tile([P, 128], BF, name=f"t1{qh}{h}{qc}", tag="t1")
                    T2 = ttp.tile([P, 128], BF, name=f"t2{qh}{h}{qc}", tag="t2")
                    nc.sync.dma_start(T1[:], Ut[:, 0:128], transpose=True)
                    # T2 rows 64:96 = Ut cols 128:160 (rowsum, d2b)
                    nc.sync.dma_start(T2[:], Ut[:, 64:192], transpose=True)
                    Th.append((T1, T2))

            def out_block(qh, h, qcs=range(4), drain=True):
                qsl = ds(512 * qh, 512)
                if h == 0 and qh not in OUTPs:
                    OUTPs[qh] = [
                        ppO.tile([P, 512], F32, name=f"op{qh}{d}", tag="po")
                        for d in range(3)
                    ]
                OUTP = OUTPs[qh]
                first, last = h == 0, h == NH - 1
                for qc in qcs:
                    T1, T2 = Ts[(qh, h)][qc]
                    st = first and qc == 0
                    sp = last and qc == 3
                    col = ts(qc, P)
                    for d in range(2):
                        nc.tensor.matmul(
                            OUTP[d][:, col], lhsT=WFT[:, h, d, :],
                            rhs=T1[:], start=st, stop=sp,
                        )
                    # T2 rows 32:64 = d2a feats, rows 65:81 = d2b feats, so
                    # one matmul covers the whole d2 projection
                    nc.tensor.matmul(
                        OUTP[2][:, col], lhsT=WFT[:, h, 3, :],
                        rhs=T2[:], start=st, stop=sp,
                    )
                if not drain:
                    return
                del Ts[(qh, h)]
                if h >= 1:
                    del Es[(qh, h - 1)]
                if last:
                    # drain the three output components in parallel
                    nc.scalar.copy(out=OutSB[:, 0, qsl], in_=OUTP[0][:])
                    nc.sync.dma_start(out.ap()[:, 0, qsl], OutSB[:, 0, qsl])
                    nc.vector.tensor_copy(out=OutSB[:, 1, qsl], in_=OUTP[1][:])
                    nc.scalar.dma_start(out.ap()[:, 1, qsl], OutSB[:, 1, qsl])
                    nc.vector.tensor_copy(out=OutSB[:, 2, qsl], in_=OUTP[2][:])
                    nc.sync.dma_start(out.ap()[:, 2, qsl], OutSB[:, 2, qsl])

            in_main[0] = True
            for i, (qh, h) in enumerate(units):
                if 2 <= i and q1jobs:
                    qk_job(*q1jobs.pop(0), pool_tag=(ppU, "pu"))
                score_block(qh, h)
                if i >= 1:
                    u_block(*units[i - 1])
                if i >= 2:
                    out_block(*units[i - 2])
            # tail: interleave the last u-block's q-chunks with unit 14's
            # output projections so transpose latency stays hidden
            for qc in range(4):
                u_block(*units[15], qcs=(qc,))
                out_block(*units[14], qcs=(qc,), drain=False)
            out_block(*units[14], qcs=())
            out_block(*units[15])

    nc.compile()
    return nc


def _prep_inputs(vn_x, Wq, Wk, Wv, Wo):
    bf = ml_dtypes.bfloat16
    WoT = np.ascontiguousarray(np.asarray(Wo, np.float32).T)  # (384, 128)
    wf = np.zeros((P, NH, 4, P), np.float32)
    for h in range(NH):
        blk = WoT[FH * h : FH * h + FH]  # (48, 128)
        wf[0:48, h, 0] = blk
        wf[48:96, h, 1] = blk
        # T2 = transpose of Ut cols 64:192: rows 32:64 = d2a feats (Ut cols
        # 96:128), row 64 = normalized row-sum, rows 65:81 = d2b feats
        wf[32:64, h, 3] = blk[0:32]
        wf[65:81, h, 3] = blk[32:48]
    def pad_pairs(W):
        Wt = np.asarray(W, np.float32).T  # (128, 384)
        arr = np.zeros((P, 4, 2, 64), np.float32)
        for h in range(NH):
            arr[:, h // 2, h % 2, 0:FH] = Wt[:, FH * h : FH * h + FH]
        return arr.reshape(P, 512)

    wkt = pad_pairs(Wk)
    wqt = pad_pairs(Wq)
    wvt = np.asarray(Wv, np.float32).T
    x = np.asarray(vn_x, np.float32)  # (B, C, 3, N)
    maps = []
    for b in range(NCORES):
        xb = x[b]
        packed = np.concatenate(
            [wkt, xb[:, 0], wqt, wvt, xb[:, 1], xb[:, 2],
             wf.reshape(P, NH * 4 * P)],
            axis=1,
        ).astype(bf)
        assert packed.shape == (P, PACKW)
        maps.append({"packed": np.ascontiguousarray(packed)})
    return maps


_CACHED_NC = None


def kernel(vn_x, Wq, Wk, Wv, Wo):
    global _CACHED_NC
    if _CACHED_NC is None:
        _CACHED_NC = _build_program()
    nc = _CACHED_NC
    in_maps = _prep_inputs(vn_x, Wq, Wk, Wv, Wo)
    res = run_bass_kernel_spmd(nc, in_maps, core_ids=list(range(NCORES)))
    return np.stack([res.results[b]["out"] for b in range(NCORES)])


# revision 7
# speedup vs baseline: 1.3523x; 1.0220x over previous
"""Trainium2 Bass kernel v2 for nn_Attention_47313359733175.

Vector-neuron attention: B=8, C=128, N=1024, H=8 heads, head dim 144.
Data-parallel over batch; core b computes batch b.

Differences vs the prior baseline (199392 ns):
  - Scores use 2 matmuls per (head, key-chunk, q-half) tile instead of 3:
    Q/K are packed per head into a 128-row tile (d0|d1 feats + 32 zero pad
    rows) and a 64-row tile (d2 feats + 16 zero pad rows).  Pad rows are
    zeroed by DMA from a zeros input.  PE: 196608 -> 131072 cycles.
  - Projections run on dense 3x128-chunk weights (VNLinear weights are
    shared across the 3 vector components, so Wk.T / Wq.T DMA once).
  - All SBUF operands are bf16 (psum stays fp32): halves DMA + SBUF.
  - exp runs once per m-pair over a [128,1024] 2-bank psum tile.
"""

import sys

sys.path.insert(0, "/opt/trn_rl_repo")

import numpy as np
import ml_dtypes
from contextlib import ExitStack

import concourse.bass as bass
import concourse.bacc as bacc
import concourse.mybir as mybir
import concourse.tile as tile
from concourse.bass import ts, ds
from concourse.bass_utils import run_bass_kernel_spmd

P = 128
N = 1024
C = 128
F = 384
NH = 8
FH = 48
D3 = 3
SEG = 160        # V segment: [feats 0-127][ones][15 pad][feats 128-143]
VW = SEG * NH
NCORES = 8
SCALE = float(FH) ** -0.5

F32 = mybir.dt.float32
BF = mybir.dt.bfloat16

# packed layout (bf16): WKT | X0 | WQT | WVT | X1 | X2 | WFT
# WKT/WQT are head-pair padded: [128c, 4 pairs, (64+64)] with each head's 48
# output channels at a 64-aligned base (engine partition ranges must start
# 32-aligned, so scatter copies move whole 64-row blocks).
O_WKT = 0
O_X0 = 512
O_WQT = O_X0 + N
O_WVT = O_WQT + 512
O_X1 = O_WVT + 384
O_X2 = O_X1 + N
O_WFT = O_X2 + N
PACKW = O_WFT + NH * 4 * P  # 4480 + 4096 = 8576


def _build_program():
    nc = bacc.Bacc(
        "TRN2", target_bir_lowering=False, debug=False, enable_asserts=False
    )

    packed = nc.dram_tensor("packed", (P, PACKW), BF, kind="ExternalInput")
    out = nc.dram_tensor("out", (C, D3, N), F32, kind="ExternalOutput")

    with tile.TileContext(nc) as tc:
        with ExitStack() as ctx:
            const = ctx.enter_context(tc.tile_pool(name="const", bufs=1))
            vpool = ctx.enter_context(tc.tile_pool(name="vpool", bufs=1))
            qkp = ctx.enter_context(tc.tile_pool(name="qkp", bufs=1))
            epool = ctx.enter_context(tc.tile_pool(name="epool", bufs=20))
            utp = ctx.enter_context(tc.tile_pool(name="utp", bufs=8))
            ttp = ctx.enter_context(tc.tile_pool(name="ttp", bufs=12))
            rrp = ctx.enter_context(tc.tile_pool(name="rrp", bufs=8))
            # PSUM: 2 + 3 + 3 = 8 banks
            ppBig = ctx.enter_context(tc.tile_pool(name="ppBig", bufs=2, space="PSUM"))
            ppU = ctx.enter_context(tc.tile_pool(name="ppU", bufs=3, space="PSUM"))
            ppO = ctx.enter_context(tc.tile_pool(name="ppO", bufs=3, space="PSUM"))

            PK = const.tile([P, PACKW], BF, name="PK")
            cuts = [0, O_X0, O_X0 + 512, O_WQT, O_WVT, O_X1, O_X2, O_WFT, PACKW]
            for ci in range(len(cuts) - 1):
                nc.sync.dma_start(
                    PK[:, cuts[ci] : cuts[ci + 1]],
                    packed.ap()[:, cuts[ci] : cuts[ci + 1]],
                )
            WKT = PK[:, O_WKT : O_WKT + 512]
            WQT = PK[:, O_WQT : O_WQT + 512]
            WVT = PK[:, O_WVT : O_WVT + F]
            Xd = [
                PK[:, O_X0 : O_X0 + N],
                PK[:, O_X1 : O_X1 + N],
                PK[:, O_X2 : O_X2 + N],
            ]
            WFT = PK[:, O_WFT:PACKW].rearrange("p (h j q) -> p h j q", h=NH, j=4)
            OutSB = const.tile([P, D3, N], F32, name="OutSB")

            # Q/K packed score operands (per head): A [128 rows: d0 48+16z |
            # d1 48+16z], B [64 rows: d2 48+16z]
            AK = qkp.tile([P, NH, N], BF, name="AK")
            AQ = qkp.tile([P, NH, N], BF, name="AQ")
            BK = qkp.tile([64, NH, N], BF, name="BK")
            BQ = qkp.tile([64, NH, N], BF, name="BQ")
            # pad rows (48:64, 112:128) are zero because the padded weight
            # rows produce zero psum that the 64-block copies carry along

            Vseq = [
                vpool.tile([P, VW], BF, name=f"vs{m}", tag=f"vs{m}")
                for m in range(8)
            ]
            vrs = []
            # prologue rotation: DVE 2/5, Pool 2/5, Act 1/5 (Act owns exp);
            # main-loop weave jobs avoid Act entirely.
            # psum-sourced copies may only run on DVE/Act (GPSIMD cannot
            # access PSUM).  Pre-main Act is exp-free so copies split 50/50;
            # in the main loop exp loads Act heavily, so copies lean DVE.
            engs_pro = [nc.vector, nc.scalar]
            engs_main = [nc.vector, nc.vector, nc.scalar]
            in_main = [False]
            ei = [0]

            def rot_copy(out_, in_, main=False):
                es = engs_main if in_main[0] else engs_pro
                e = es[ei[0] % len(es)]
                ei[0] += 1
                if e is nc.scalar:
                    e.copy(out=out_, in_=in_)
                else:
                    e.tensor_copy(out=out_, in_=in_)

            # Vseq segment layout (v3): [feats-a 0:128 | ones@128 | d2b
            # 129:145 | pad 145:160].  The U-form matmul streams cols 0:145
            # so one mm yields features, row-sum, and the d2 leftovers.
            for m in range(8):
                vrs.append(Vseq[m].rearrange("p (h s) -> p h s", s=SEG))
                vu = Vseq[m].bitcast(mybir.dt.uint16).rearrange(
                    "p (h s) -> p h s", s=SEG
                )
                nc.vector.memset(vu[:, :, 128], 0x3F80)

            pools3 = [(ppU, "pu"), (ppO, "po"), (ppBig, "ps")]
            pidx = [0]

            def next_pool():
                p_ = pools3[pidx[0] % len(pools3)]
                pidx[0] += 1
                return p_

            # ---- projection jobs ----
            def v_job(m, d, pool_tag=None, main=False):
                pool, tag = pool_tag or next_pool()
                pv = pool.tile([P, F], F32, name=f"pv{m}{d}", tag=tag)
                nc.tensor.matmul(
                    pv[:], lhsT=Xd[d][:, ts(m, P)], rhs=WVT[:],
                    start=True, stop=True,
                )
                pvh = pv.rearrange("p (h f) -> p h f", f=FH)
                vr = vrs[m]
                if d < 2:
                    rot_copy(vr[:, :, 48 * d : 48 * d + 48], pvh, main)
                else:
                    rot_copy(vr[:, :, 96:128], pvh[:, :, 0:32], main)
                    rot_copy(vr[:, :, 129:145], pvh[:, :, 32:48], main)

            def qk_job(which, d, j, half, pool_tag=None, main=False):
                # j is the head pair; chunk rows = [head 2j (48+16z) | head
                # 2j+1 (48+16z)] -> 64-aligned block copies into A/B tiles
                WT, A, B = (WKT, AK, BK) if which == "k" else (WQT, AQ, BQ)
                pool, tag = pool_tag or next_pool()
                pk = pool.tile([P, 512], F32, name=f"p{which}{d}{j}{half}", tag=tag)
                nc.tensor.matmul(
                    pk[:], lhsT=WT[:, ts(j, P)], rhs=Xd[d][:, ts(half, 512)],
                    start=True, stop=True,
                )
                sl = ts(half, 512)
                for i in range(2):
                    src = pk[64 * i : 64 * i + 64, :]
                    if d < 2:
                        rot_copy(A[64 * d : 64 * d + 64, 2 * j + i, sl], src, main)
                    else:
                        rot_copy(B[0:64, 2 * j + i, sl], src, main)

            # upfront: only what unit 0/1 needs -- K pairs 0-1 (both
            # halves), Q half-0 pairs 0-1, V for keys 0:768.  Everything
            # else is woven into the main loop ahead of its deadline.
            jobs = []
            for j in range(2):
                for d in range(D3):
                    jobs.append(("k", d, j, 0))
                    jobs.append(("k", d, j, 1))
                    jobs.append(("q", d, j, 0))
            vjobs = [(m, d) for m in range(4) for d in range(D3)]
            wove = []
            vi = 0
            for i, jb in enumerate(jobs):
                wove.append(("qk", jb))
                if i % 2 == 1 and vi < len(vjobs):
                    wove.append(("v", vjobs[vi])); vi += 1
            while vi < len(vjobs):
                wove.append(("v", vjobs[vi])); vi += 1
            for kind, jb in wove:
                if kind == "qk":
                    qk_job(*jb)
                else:
                    v_job(*jb)
            # woven into the first two score blocks via the (still free) ppO
            # ring: remaining V, then K/Q0 pairs 2-3 (deadlines i=4/i=6)
            vweave = [("v", m, d) for m in range(4, 8) for d in range(D3)]
            for j in (2, 3):
                for d in range(D3):
                    vweave.append(("k", d, j, 0))
                    vweave.append(("k", d, j, 1))
                    vweave.append(("q", d, j, 0))
            # woven one-per-unit from i=2: Q half-1 (pair j needed at i=8+2j)
            q1jobs = [("q", d, j, 1) for j in range(4) for d in range(D3)]
            # ---- main loop (U-form) ----
            # per head: scores+exp; then the previous head's U block
            # (U-matmuls -> recip -> normalize-mul -> XBAR transposes); the
            # head-before-that's output projection (so transpose DMA latency
            # is fully hidden).
            ExpF = mybir.ActivationFunctionType.Exp
            mulengs = [nc.vector, nc.vector, nc.vector, nc.scalar]
            units = [(qh, h) for qh in range(2) for h in range(NH)]
            Es = {}
            Ts = {}
            OUTPs = {}

            def score_block(qh, h):
                qsl = ds(512 * qh, 512)
                Eh = []
                for m in range(8):
                    for _ in range(2):
                        if qh == 0 and h < 2 and vweave:
                            jb = vweave.pop(0)
                            if jb[0] == "v":
                                v_job(jb[1], jb[2], pool_tag=(ppO, "po"))
                            else:
                                qk_job(*jb, pool_tag=(ppO, "po"))
                    psS = ppBig.tile([P, 512], F32, name=f"s{qh}{h}{m}", tag="ps")
                    nc.tensor.matmul(
                        psS[:],
                        lhsT=AK[:, h, ts(m, P)], rhs=AQ[:, h, qsl],
                        start=True, stop=False,
                    )
                    nc.tensor.matmul(
                        psS[:],
                        lhsT=BK[0:64, h, ts(m, P)], rhs=BQ[0:64, h, qsl],
                        start=False, stop=True,
                    )
                    E = epool.tile([P, 512], BF, name=f"e{qh}{h}{m}", tag="e")
                    nc.scalar.activation(E[:], psS[:], ExpF, scale=SCALE)
                    Eh.append(E)
                Es[(qh, h)] = Eh

            def u_block(qh, h, qcs=range(4)):
                Eh = Es[(qh, h)]
                Th = Ts.setdefault((qh, h), [])
                for qc in qcs:
                    U = ppU.tile([P, 160], F32, name=f"u{qh}{h}{qc}", tag="pu")
                    for m in range(8):
                        nc.tensor.matmul(
                            U[:, 0:145],
                            lhsT=Eh[m][:, ts(qc, P)],
                            rhs=Vseq[m][:, SEG * h : SEG * h + 145],
                            start=(m == 0), stop=(m == 7),
                        )
                    rr = rrp.tile([P, 1], F32, name=f"rr{qh}{h}{qc}", tag="rr")
                    nc.vector.reciprocal(out=rr[:], in_=U[:, 128:129])
                    Ut = utp.tile([P, 192], BF, name=f"ut{qh}{h}{qc}", tag="ut")
                    # cols 145:192 feed the second XBAR transpose; zero them
                    # (gpsimd is idle) so its input stays finite
                    nc.gpsimd.memset(Ut[:, 145:192], 0)
                    eng = mulengs[(4 * h + qc) % len(mulengs)]
                    if eng is nc.scalar:
                        eng.activation(
                            Ut[:, 0:145], U[:, 0:145],
                            mybir.ActivationFunctionType.Copy, scale=rr,
                        )
                    else:
                        eng.tensor_scalar_mul(Ut[:, 0:145], U[:, 0:145], rr)
                    T1 = ttp.tile([P, 128], BF, name=f"t1{qh}{h}{qc}", tag="t1")
                    T2 = ttp.tile([P, 128], BF, name=f"t2{qh}{h}{qc}", tag="t2")
                    nc.sync.dma_start(T1[:], Ut[:, 0:128], transpose=True)
                    # T2 rows 64:96 = Ut cols 128:160 (rowsum, d2b)
                    nc.sync.dma_start(T2[:], Ut[:, 64:192], transpose=True)
                    Th.append((T1, T2))

            def out_block(qh, h, qcs=range(4), drain=True):
                qsl = ds(512 * qh, 512)
                if h == 0 and qh not in OUTPs:
                    OUTPs[qh] = [
                        ppO.tile([P, 512], F32, name=f"op{qh}{d}", tag="po")
                        for d in range(3)
                    ]
                OUTP = OUTPs[qh]
                first, last = h == 0, h == NH - 1
                for qc in qcs:
                    T1, T2 = Ts[(qh, h)][qc]
                    st = first and qc == 0
                    sp = last and qc == 3
                    col = ts(qc, P)
                    for d in range(2):
                        nc.tensor.matmul(
                            OUTP[d][:, col], lhsT=WFT[:, h, d, :],
                            rhs=T1[:], start=st, stop=sp,
                        )
                    # T2 rows 32:64 = d2a feats, rows 65:81 = d2b feats, so
                    # one matmul covers the whole d2 projection
                    nc.tensor.matmul(
                        OUTP[2][:, col], lhsT=WFT[:, h, 3, :],
                        rhs=T2[:], start=st, stop=sp,
                    )
                if not drain:
                    return
                del Ts[(qh, h)]
                if h >= 1:
                    del Es[(qh, h - 1)]
                if last:
                    # drain the three output components in parallel
                    nc.scalar.copy(out=OutSB[:, 0, qsl], in_=OUTP[0][:])
                    nc.sync.dma_start(out.ap()[:, 0, qsl], OutSB[:, 0, qsl])
                    nc.vector.tensor_copy(out=OutSB[:, 1, qsl], in_=OUTP[1][:])
                    nc.scalar.dma_start(out.ap()[:, 1, qsl], OutSB[:, 1, qsl])
                    nc.vector.tensor_copy(out=OutSB[:, 2, qsl], in_=OUTP[2][:])
                    nc.sync.dma_start(out.ap()[:, 2, qsl], OutSB[:, 2, qsl])

            in_main[0] = True
            for i, (qh, h) in enumerate(units):
                if 2 <= i and q1jobs:
                    qk_job(*q1jobs.pop(0), pool_tag=(ppU, "pu"))
                score_block(qh, h)
                if i >= 1:
                    u_block(*units[i - 1])
                if i >= 2:
                    out_block(*units[i - 2])
            # tail: interleave the last u-block's q-chunks with unit 14's
            # output projections so transpose latency stays hidden
            for qc in range(4):
                u_block(*units[15], qcs=(qc,))
                out_block(*units[14], qcs=(qc,), drain=False)
            out_block(*units[14], qcs=())
            out_block(*units[15])

    nc.compile()
    return nc


def _prep_inputs(vn_x, Wq, Wk, Wv, Wo):
    bf = ml_dtypes.bfloat16
    WoT = np.ascontiguousarray(np.asarray(Wo, np.float32).T)  # (384, 128)
    wf = np.zeros((P, NH, 4, P), np.float32)
    for h in range(NH):
        blk = WoT[FH * h : FH * h + FH]  # (48, 128)
        wf[0:48, h, 0] = blk
        wf[48:96, h, 1] = blk
        # T2 = transpose of Ut cols 64:192: rows 32:64 = d2a feats (Ut cols
        # 96:128), row 64 = normalized row-sum, rows 65:81 = d2b feats
        wf[32:64, h, 3] = blk[0:32]
        wf[65:81, h, 3] = blk[32:48]
    def pad_pairs(W):
        Wt = np.asarray(W, np.float32).T  # (128, 384)
        arr = np.zeros((P, 4, 2, 64), np.float32)
        for h in range(NH):
            arr[:, h // 2, h % 2, 0:FH] = Wt[:, FH * h : FH * h + FH]
        return arr.reshape(P, 512)

    wkt = pad_pairs(Wk)
    wqt = pad_pairs(Wq)
    wvt = np.asarray(Wv, np.float32).T
    x = np.asarray(vn_x, np.float32)  # (B, C, 3, N)
    maps = []
    for b in range(NCORES):
        xb = x[b]
        packed = np.concatenate(
            [wkt, xb[:, 0], wqt, wvt, xb[:, 1], xb[:, 2],
             wf.reshape(P, NH * 4 * P)],
            axis=1,
        ).astype(bf)
        assert packed.shape == (P, PACKW)
        maps.append({"packed": np.ascontiguousarray(packed)})
    return maps


_CACHED_NC = None


def kernel(vn_x, Wq, Wk, Wv, Wo):
    global _CACHED_NC
    if _CACHED_NC is None:
        _CACHED_NC = _build_program()
    nc = _CACHED_NC
    in_maps = _prep_inputs(vn_x, Wq, Wk, Wv, Wo)
    res = run_bass_kernel_spmd(nc, in_maps, core_ids=list(range(NCORES)))
    return np.stack([res.results[b]["out"] for b in range(NCORES)])


# revision 8
# speedup vs baseline: 1.3802x; 1.0207x over previous
"""Trainium2 Bass kernel v3 for nn_Attention_47313359733175.

Vector-neuron attention: B=8, C=128, N=1024, H=8 heads, head dim 144.
Data-parallel over batch; core b computes batch b; no collectives.

Design (vs the 199392 ns predecessor):
  - All SBUF operands are bf16 (psum fp32): halves DMA/SBUF, and bf16
    matmuls run 1 cycle/column like fp32r.
  - Scores: 2 matmuls per (head, key-chunk, q-half) tile instead of 3.
    Q/K live in per-head packed tiles: A[128 rows: d0(48)+pad | d1(48)+pad]
    and B[64 rows: d2(48)+pad], built by 64-aligned block copies out of
    head-pair-padded projection chunks (the weight padding provides the
    zero pad rows; engine partition ranges must start 32-aligned).
  - attn@V uses the U-form out[q,feat] (145 free cols: 144 feats + a ones
    column that yields softmax row-sums per PARTITION), so normalization
    is a per-partition reciprocal + tensor_scalar_mul -- no gpsimd
    partition broadcast.  74K PE cycles vs 131K for the U^T form.
  - Normalized tiles are transposed back to [feat,q] by the DMA XBAR
    (free on idle DMA engines); the d2 output projection reads the second
    transpose directly (d2a at rows 32:64, d2b at rows 65:81), so the
    whole output projection is 3 matmuls per q-chunk.
  - Flat software pipeline over the 16 (q-half, head) units: scores+exp
    of unit i interleave with unit i-1's attn@V matmuls (the Act exp
    chain outpaces score matmul pairs, so the woven U work fills the psS
    ring stalls) and unit i-2's output projection (hides XBAR latency).
    Projection work beyond what unit 0/1 needs is woven into the early
    units; exp owns Act, so copies lean DVE inside the main loop.
"""

import sys

sys.path.insert(0, "/opt/trn_rl_repo")

import numpy as np
import ml_dtypes
from contextlib import ExitStack

import concourse.bass as bass
import concourse.bacc as bacc
import concourse.mybir as mybir
import concourse.tile as tile
from concourse.bass import ts, ds
from concourse.bass_utils import run_bass_kernel_spmd

P = 128
N = 1024
C = 128
F = 384
NH = 8
FH = 48
D3 = 3
SEG = 160        # V segment: [feats 0-127][ones][15 pad][feats 128-143]
VW = SEG * NH
NCORES = 8
SCALE = float(FH) ** -0.5

F32 = mybir.dt.float32
BF = mybir.dt.bfloat16

# packed layout (bf16): WKT | X0 | WQT | WVT | X1 | X2 | WFT
# WKT/WQT are head-pair padded: [128c, 4 pairs, (64+64)] with each head's 48
# output channels at a 64-aligned base (engine partition ranges must start
# 32-aligned, so scatter copies move whole 64-row blocks).
O_WKT = 0
O_X0 = 512
O_WQT = O_X0 + N
O_WVT = O_WQT + 512
O_X1 = O_WVT + 384
O_X2 = O_X1 + N
O_WFT = O_X2 + N
PACKW = O_WFT + NH * 4 * P  # 4480 + 4096 = 8576


def _build_program():
    nc = bacc.Bacc(
        "TRN2", target_bir_lowering=False, debug=False, enable_asserts=False
    )

    packed = nc.dram_tensor("packed", (P, PACKW), BF, kind="ExternalInput")
    out = nc.dram_tensor("out", (C, D3, N), F32, kind="ExternalOutput")

    with tile.TileContext(nc) as tc:
        with ExitStack() as ctx:
            const = ctx.enter_context(tc.tile_pool(name="const", bufs=1))
            vpool = ctx.enter_context(tc.tile_pool(name="vpool", bufs=1))
            qkp = ctx.enter_context(tc.tile_pool(name="qkp", bufs=1))
            epool = ctx.enter_context(tc.tile_pool(name="epool", bufs=20))
            utp = ctx.enter_context(tc.tile_pool(name="utp", bufs=8))
            ttp = ctx.enter_context(tc.tile_pool(name="ttp", bufs=12))
            rrp = ctx.enter_context(tc.tile_pool(name="rrp", bufs=8))
            # PSUM: 2 + 3 + 3 = 8 banks
            ppBig = ctx.enter_context(tc.tile_pool(name="ppBig", bufs=3, space="PSUM"))
            ppU = ctx.enter_context(tc.tile_pool(name="ppU", bufs=2, space="PSUM"))
            ppO = ctx.enter_context(tc.tile_pool(name="ppO", bufs=3, space="PSUM"))

            PK = const.tile([P, PACKW], BF, name="PK")
            cuts = [0, O_X0, O_X0 + 512, O_WQT, O_WVT, O_X1, O_X2, O_WFT, PACKW]
            for ci in range(len(cuts) - 1):
                nc.sync.dma_start(
                    PK[:, cuts[ci] : cuts[ci + 1]],
                    packed.ap()[:, cuts[ci] : cuts[ci + 1]],
                )
            WKT = PK[:, O_WKT : O_WKT + 512]
            WQT = PK[:, O_WQT : O_WQT + 512]
            WVT = PK[:, O_WVT : O_WVT + F]
            Xd = [
                PK[:, O_X0 : O_X0 + N],
                PK[:, O_X1 : O_X1 + N],
                PK[:, O_X2 : O_X2 + N],
            ]
            WFT = PK[:, O_WFT:PACKW].rearrange("p (h j q) -> p h j q", h=NH, j=4)
            OutSB = const.tile([P, D3, N], F32, name="OutSB")

            # Q/K packed score operands (per head): A [128 rows: d0 48+16z |
            # d1 48+16z], B [64 rows: d2 48+16z]
            AK = qkp.tile([P, NH, N], BF, name="AK")
            AQ = qkp.tile([P, NH, N], BF, name="AQ")
            BK = qkp.tile([64, NH, N], BF, name="BK")
            BQ = qkp.tile([64, NH, N], BF, name="BQ")
            # pad rows (48:64, 112:128) are zero because the padded weight
            # rows produce zero psum that the 64-block copies carry along

            Vseq = [
                vpool.tile([P, VW], BF, name=f"vs{m}", tag=f"vs{m}")
                for m in range(8)
            ]
            vrs = []
            # prologue rotation: DVE 2/5, Pool 2/5, Act 1/5 (Act owns exp);
            # main-loop weave jobs avoid Act entirely.
            # psum-sourced copies may only run on DVE/Act (GPSIMD cannot
            # access PSUM).  Pre-main Act is exp-free so copies split 50/50;
            # in the main loop exp loads Act heavily, so copies lean DVE.
            engs_pro = [nc.vector, nc.scalar]
            engs_main = [nc.vector, nc.vector, nc.scalar]
            in_main = [False]
            ei = [0]

            def rot_copy(out_, in_, main=False):
                es = engs_main if in_main[0] else engs_pro
                e = es[ei[0] % len(es)]
                ei[0] += 1
                if e is nc.scalar:
                    e.copy(out=out_, in_=in_)
                else:
                    e.tensor_copy(out=out_, in_=in_)

            # Vseq segment layout (v3): [feats-a 0:128 | ones@128 | d2b
            # 129:145 | pad 145:160].  The U-form matmul streams cols 0:145
            # so one mm yields features, row-sum, and the d2 leftovers.
            for m in range(8):
                vrs.append(Vseq[m].rearrange("p (h s) -> p h s", s=SEG))
                vu = Vseq[m].bitcast(mybir.dt.uint16).rearrange(
                    "p (h s) -> p h s", s=SEG
                )
                nc.vector.memset(vu[:, :, 128], 0x3F80)

            pools3 = [(ppU, "pu"), (ppO, "po"), (ppBig, "ps")]
            pidx = [0]

            def next_pool():
                p_ = pools3[pidx[0] % len(pools3)]
                pidx[0] += 1
                return p_

            # ---- projection jobs ----
            def v_job(m, d, pool_tag=None, main=False):
                pool, tag = pool_tag or next_pool()
                pv = pool.tile([P, F], F32, name=f"pv{m}{d}", tag=tag)
                nc.tensor.matmul(
                    pv[:], lhsT=Xd[d][:, ts(m, P)], rhs=WVT[:],
                    start=True, stop=True,
                )
                pvh = pv.rearrange("p (h f) -> p h f", f=FH)
                vr = vrs[m]
                if d < 2:
                    rot_copy(vr[:, :, 48 * d : 48 * d + 48], pvh, main)
                else:
                    rot_copy(vr[:, :, 96:128], pvh[:, :, 0:32], main)
                    rot_copy(vr[:, :, 129:145], pvh[:, :, 32:48], main)

            def qk_job(which, d, j, half, pool_tag=None, main=False):
                # j is the head pair; chunk rows = [head 2j (48+16z) | head
                # 2j+1 (48+16z)] -> 64-aligned block copies into A/B tiles
                WT, A, B = (WKT, AK, BK) if which == "k" else (WQT, AQ, BQ)
                pool, tag = pool_tag or next_pool()
                pk = pool.tile([P, 512], F32, name=f"p{which}{d}{j}{half}", tag=tag)
                nc.tensor.matmul(
                    pk[:], lhsT=WT[:, ts(j, P)], rhs=Xd[d][:, ts(half, 512)],
                    start=True, stop=True,
                )
                sl = ts(half, 512)
                for i in range(2):
                    src = pk[64 * i : 64 * i + 64, :]
                    if d < 2:
                        rot_copy(A[64 * d : 64 * d + 64, 2 * j + i, sl], src, main)
                    else:
                        rot_copy(B[0:64, 2 * j + i, sl], src, main)

            # upfront: only what unit 0/1 needs -- K pairs 0-1 (both
            # halves), Q half-0 pairs 0-1, V for keys 0:768.  Everything
            # else is woven into the main loop ahead of its deadline.
            jobs = []
            for j in range(2):
                for d in range(D3):
                    jobs.append(("k", d, j, 0))
                    jobs.append(("k", d, j, 1))
                    jobs.append(("q", d, j, 0))
            vjobs = [(m, d) for m in range(4) for d in range(D3)]
            wove = []
            vi = 0
            for i, jb in enumerate(jobs):
                wove.append(("qk", jb))
                if i % 2 == 1 and vi < len(vjobs):
                    wove.append(("v", vjobs[vi])); vi += 1
            while vi < len(vjobs):
                wove.append(("v", vjobs[vi])); vi += 1
            for kind, jb in wove:
                if kind == "qk":
                    qk_job(*jb)
                else:
                    v_job(*jb)
            # woven into the first two score blocks via the (still free) ppO
            # ring: remaining V, then K/Q0 pairs 2-3 (deadlines i=4/i=6)
            vweave = [("v", m, d) for m in range(4, 8) for d in range(D3)]
            for j in (2, 3):
                for d in range(D3):
                    vweave.append(("k", d, j, 0))
                    vweave.append(("k", d, j, 1))
                    vweave.append(("q", d, j, 0))
            # woven one-per-unit from i=2: Q half-1 (pair j needed at i=8+2j)
            q1jobs = [("q", d, j, 1) for j in range(4) for d in range(D3)]
            # ---- main loop (U-form) ----
            # per head: scores+exp; then the previous head's U block
            # (U-matmuls -> recip -> normalize-mul -> XBAR transposes); the
            # head-before-that's output projection (so transpose DMA latency
            # is fully hidden).
            ExpF = mybir.ActivationFunctionType.Exp
            mulengs = [nc.vector, nc.vector, nc.vector, nc.scalar]
            units = [(qh, h) for qh in range(2) for h in range(NH)]
            Es = {}
            Ts = {}
            OUTPs = {}

            Us = {}

            def u_qc_mms(qh, h, qc, m0, m1):
                Eh = Es[(qh, h)]
                if m0 == 0:
                    Us[(qh, h, qc)] = ppU.tile(
                        [P, 160], F32, name=f"u{qh}{h}{qc}", tag="pu"
                    )
                U = Us[(qh, h, qc)]
                for m in range(m0, m1):
                    nc.tensor.matmul(
                        U[:, 0:145],
                        lhsT=Eh[m][:, ts(qc, P)],
                        rhs=Vseq[m][:, SEG * h : SEG * h + 145],
                        start=(m == 0), stop=(m == 7),
                    )

            def u_qc_finish(qh, h, qc):
                U = Us.pop((qh, h, qc))
                rr = rrp.tile([P, 1], F32, name=f"rr{qh}{h}{qc}", tag="rr")
                nc.vector.reciprocal(out=rr[:], in_=U[:, 128:129])
                Ut = utp.tile([P, 192], BF, name=f"ut{qh}{h}{qc}", tag="ut")
                # cols 145:192 feed the second XBAR transpose; zero them
                # (gpsimd is idle) so its input stays finite
                nc.gpsimd.memset(Ut[:, 145:192], 0)
                eng = mulengs[(4 * h + qc) % len(mulengs)]
                if eng is nc.scalar:
                    eng.activation(
                        Ut[:, 0:145], U[:, 0:145],
                        mybir.ActivationFunctionType.Copy, scale=rr,
                    )
                else:
                    eng.tensor_scalar_mul(Ut[:, 0:145], U[:, 0:145], rr)
                T1 = ttp.tile([P, 128], BF, name=f"t1{qh}{h}{qc}", tag="t1")
                T2 = ttp.tile([P, 128], BF, name=f"t2{qh}{h}{qc}", tag="t2")
                nc.sync.dma_start(T1[:], Ut[:, 0:128], transpose=True)
                # T2 rows 64:96 = Ut cols 128:160 (rowsum, d2b)
                nc.sync.dma_start(T2[:], Ut[:, 64:192], transpose=True)
                Ts.setdefault((qh, h), []).append((T1, T2))

            def score_block(qh, h, prev=None):
                # the previous unit's attn@V matmuls interleave into the
                # score m-loop: the Act exp chain (570ns) outpaces a score
                # mm-pair (426ns), so pure score streams stall on the psS
                # ring -- the woven U work fills those slots.
                qsl = ds(512 * qh, 512)
                Eh = []
                for m in range(8):
                    for _ in range(2):
                        if qh == 0 and h < 2 and vweave:
                            jb = vweave.pop(0)
                            if jb[0] == "v":
                                v_job(jb[1], jb[2], pool_tag=(ppO, "po"))
                            else:
                                qk_job(*jb, pool_tag=(ppO, "po"))
                    psS = ppBig.tile([P, 512], F32, name=f"s{qh}{h}{m}", tag="ps")
                    nc.tensor.matmul(
                        psS[:],
                        lhsT=AK[:, h, ts(m, P)], rhs=AQ[:, h, qsl],
                        start=True, stop=False,
                    )
                    nc.tensor.matmul(
                        psS[:],
                        lhsT=BK[0:64, h, ts(m, P)], rhs=BQ[0:64, h, qsl],
                        start=False, stop=True,
                    )
                    E = epool.tile([P, 512], BF, name=f"e{qh}{h}{m}", tag="e")
                    nc.scalar.activation(E[:], psS[:], ExpF, scale=SCALE)
                    Eh.append(E)
                    if prev is not None:
                        qc = m // 2
                        if m % 2 == 0:
                            u_qc_mms(*prev, qc, 0, 4)
                        else:
                            u_qc_mms(*prev, qc, 4, 8)
                            u_qc_finish(*prev, qc)
                Es[(qh, h)] = Eh

            def u_block(qh, h, qcs=range(4)):
                for qc in qcs:
                    u_qc_mms(qh, h, qc, 0, 8)
                    u_qc_finish(qh, h, qc)

            def out_block(qh, h, qcs=range(4), drain=True):
                qsl = ds(512 * qh, 512)
                if h == 0 and qh not in OUTPs:
                    OUTPs[qh] = [
                        ppO.tile([P, 512], F32, name=f"op{qh}{d}", tag="po")
                        for d in range(3)
                    ]
                OUTP = OUTPs[qh]
                first, last = h == 0, h == NH - 1
                for qc in qcs:
                    T1, T2 = Ts[(qh, h)][qc]
                    st = first and qc == 0
                    sp = last and qc == 3
                    col = ts(qc, P)
                    for d in range(2):
                        nc.tensor.matmul(
                            OUTP[d][:, col], lhsT=WFT[:, h, d, :],
                            rhs=T1[:], start=st, stop=sp,
                        )
                    # T2 rows 32:64 = d2a feats, rows 65:81 = d2b feats, so
                    # one matmul covers the whole d2 projection
                    nc.tensor.matmul(
                        OUTP[2][:, col], lhsT=WFT[:, h, 3, :],
                        rhs=T2[:], start=st, stop=sp,
                    )
                if not drain:
                    return
                del Ts[(qh, h)]
                if h >= 1:
                    del Es[(qh, h - 1)]
                if last:
                    # drain the three output components in parallel
                    nc.scalar.copy(out=OutSB[:, 0, qsl], in_=OUTP[0][:])
                    nc.sync.dma_start(out.ap()[:, 0, qsl], OutSB[:, 0, qsl])
                    nc.vector.tensor_copy(out=OutSB[:, 1, qsl], in_=OUTP[1][:])
                    nc.scalar.dma_start(out.ap()[:, 1, qsl], OutSB[:, 1, qsl])
                    nc.vector.tensor_copy(out=OutSB[:, 2, qsl], in_=OUTP[2][:])
                    nc.sync.dma_start(out.ap()[:, 2, qsl], OutSB[:, 2, qsl])

            in_main[0] = True
            for i, (qh, h) in enumerate(units):
                if 2 <= i and q1jobs:
                    qk_job(*q1jobs.pop(0), pool_tag=(ppU, "pu"))
                score_block(qh, h, prev=units[i - 1] if i >= 1 else None)
                if i >= 2:
                    out_block(*units[i - 2])
            # tail: interleave the last u-block's q-chunks with unit 14's
            # output projections so transpose latency stays hidden
            for qc in range(4):
                u_block(*units[15], qcs=(qc,))
                out_block(*units[14], qcs=(qc,), drain=False)
            out_block(*units[14], qcs=())
            out_block(*units[15])

    nc.compile()
    return nc


def _prep_inputs(vn_x, Wq, Wk, Wv, Wo):
    bf = ml_dtypes.bfloat16
    WoT = np.ascontiguousarray(np.asarray(Wo, np.float32).T)  # (384, 128)
    wf = np.zeros((P, NH, 4, P), np.float32)
    for h in range(NH):
        blk = WoT[FH * h : FH * h + FH]  # (48, 128)
        wf[0:48, h, 0] = blk
        wf[48:96, h, 1] = blk
        # T2 = transpose of Ut cols 64:192: rows 32:64 = d2a feats (Ut cols
        # 96:128), row 64 = normalized row-sum, rows 65:81 = d2b feats
        wf[32:64, h, 3] = blk[0:32]
        wf[65:81, h, 3] = blk[32:48]
    def pad_pairs(W):
        Wt = np.asarray(W, np.float32).T  # (128, 384)
        arr = np.zeros((P, 4, 2, 64), np.float32)
        for h in range(NH):
            arr[:, h // 2, h % 2, 0:FH] = Wt[:, FH * h : FH * h + FH]
        return arr.reshape(P, 512)

    wkt = pad_pairs(Wk)
    wqt = pad_pairs(Wq)
    wvt = np.asarray(Wv, np.float32).T
    x = np.asarray(vn_x, np.float32)  # (B, C, 3, N)
    maps = []
    for b in range(NCORES):
        xb = x[b]
        packed = np.concatenate(
            [wkt, xb[:, 0], wqt, wvt, xb[:, 1], xb[:, 2],
             wf.reshape(P, NH * 4 * P)],
            axis=1,
        ).astype(bf)
        assert packed.shape == (P, PACKW)
        maps.append({"packed": np.ascontiguousarray(packed)})
    return maps


_CACHED_NC = None


def kernel(vn_x, Wq, Wk, Wv, Wo):
    global _CACHED_NC
    if _CACHED_NC is None:
        _CACHED_NC = _build_program()
    nc = _CACHED_NC
    in_maps = _prep_inputs(vn_x, Wq, Wk, Wv, Wo)
    res = run_bass_kernel_spmd(nc, in_maps, core_ids=list(range(NCORES)))
    return np.stack([res.results[b]["out"] for b in range(NCORES)])
